# revision 10
# baseline (speedup 1.0000x reference)
"""Cross-attention Trainium2 kernel (8-core SPMD, no collectives).

Problem: B=4, NQ=SL=D=1024, H=16, A=64.
  q = iQ @ Wq; k,v = iK @ Wkv; scores = q k^T / sqrt(A) masked; attn = softmax;
  out = (attn v) @ Wo.  Returns (out, attn).

Sharding: core c -> batch b=c//2, head-half hh=c%2 (8 heads each).
Host pre-transposes iQ/iK/mask per batch (pure data layout); the two
partial out projections per batch are summed on host.

Per-core dataflow (all matmuls float32r = full-rate fp32-reduced):
  qT[a,q] = Wq_s^T iQ^T      kT[a,s] = Wk_s^T iK^T      v[s,a] = iK Wv_s
  scoresT[s,q] psum = maskT-copy (identity matmul) + kT_h^T qT_h
  eT = Exp(0.125 * psum)                (ACT, masked entries -> 0)
  oT_ext[a+1,q] += v_ext[s,a+1]^T eT    (ones column -> softmax denoms)
  attn[q,s] = PE-transpose(eT) * recip(denom)   (normalize on PSUM evict)
  out[q,D] = oT_norm^T Wo_s
"""
import functools
import numpy as np

import concourse.bass as bass
import concourse.mybir as mybir
import concourse.tile as tile
from concourse import bacc
from concourse.bass_utils import run_bass_kernel_spmd

B, NQ, SL, D = 4, 1024, 1024, 1024
H, A = 16, 64
HPC = 8            # heads per core
HS = HPC * A       # 512 = per-core slice of hidden
N_CORES = 8
F32 = mybir.dt.float32
F32R = mybir.dt.float32r
BF16 = mybir.dt.bfloat16
MASK_NEG = -1e30


def _build_program():
    nc = bacc.Bacc("TRN2", target_bir_lowering=False, debug=False)

    iQT_h = nc.dram_tensor("iQT", [D, NQ], F32R, kind="ExternalInput")
    iKT_h = nc.dram_tensor("iKT", [D, SL], F32R, kind="ExternalInput")
    mnT_h = nc.dram_tensor("mnT", [SL, NQ], BF16, kind="ExternalInput")
    Wq_h = nc.dram_tensor("Wq_s", [D, HS], F32R, kind="ExternalInput")
    Wk_h = nc.dram_tensor("Wk_s", [D, HS], F32R, kind="ExternalInput")
    Wv_h = nc.dram_tensor("Wv_s", [D, HS], F32R, kind="ExternalInput")
    Wo_h = nc.dram_tensor("Wo_s", [HS, D], F32R, kind="ExternalInput")
    ident_h = nc.dram_tensor("ident", [128, 128], F32R, kind="ExternalInput")
    attn_h = nc.dram_tensor("attn_s", [HPC, NQ, SL], F32, kind="ExternalOutput")
    outp_h = nc.dram_tensor("out_p", [NQ, D], F32, kind="ExternalOutput")

    with tile.TileContext(nc) as tc:
        with (
            tc.tile_pool(name="big_in", bufs=1) as big_in,     # iKT then iQT [128,8,1024]
            tc.tile_pool(name="wpool", bufs=2) as wpool,       # Wk,Wv -> Wq,Wo
            tc.tile_pool(name="mpool", bufs=1) as mpool,       # maskT resident
            tc.tile_pool(name="actpool", bufs=1) as actpool,   # kT,qT,v_ext,oT resident
            tc.tile_pool(name="epool", bufs=1) as epool,       # eT per (h,qh)
            tc.tile_pool(name="stage", bufs=4) as stage,       # attn/out staging
            tc.tile_pool(name="small", bufs=2) as small,
            tc.tile_pool(name="const", bufs=1) as cpool,
            tc.tile_pool(name="dpool", bufs=2, space="DRAM") as dpool,
            tc.tile_pool(name="psA", bufs=4, space="PSUM") as psA,
            tc.tile_pool(name="psB", bufs=2, space="PSUM") as psB,
        ):
            # ---- constants
            ident_t = cpool.tile([128, 128], F32R)
            nc.sync.dma_start(out=ident_t, in_=ident_h.ap())
            ones_f32 = cpool.tile([128, 64], F32)
            nc.vector.memset(ones_f32, 1.0)
            ones_t = cpool.tile([128, 64], F32R)
            nc.vector.tensor_copy(ones_t, ones_f32)
            ident_bf = cpool.tile([128, 128], BF16)
            nc.vector.tensor_copy(ident_bf, ident_t)

            # ---- resident mask (transposed, pre-scaled by host)
            mnT_t = mpool.tile([128, 8, NQ], BF16)
            nc.sync.dma_start(
                out=mnT_t, in_=mnT_h.ap().rearrange("(o p) q -> p o q", p=128))

            # ---- phase B: kT, v from iKT
            iKT_t = big_in.tile([128, 8, SL], F32R, tag="bigin")
            nc.sync.dma_start(
                out=iKT_t, in_=iKT_h.ap().rearrange("(o p) s -> p o s", p=128))
            Wk_t = wpool.tile([128, 8, HS], F32R, tag="w")
            nc.sync.dma_start(
                out=Wk_t, in_=Wk_h.ap().rearrange("(o p) a -> p o a", p=128))
            Wv_t = wpool.tile([128, 8, HS], F32R, tag="w")
            nc.sync.dma_start(
                out=Wv_t, in_=Wv_h.ap().rearrange("(o p) a -> p o a", p=128))

            kT_t = actpool.tile([128, 4, SL], F32R)       # [(2h,a), ho, s]
            v_t = actpool.tile([128, 8, HPC, 65], F32R)   # [s_in, so, h, a+ones]
            for ao in range(4):
                for nt in range(2):
                    ps = psA.tile([128, 512], F32, tag="mm")
                    for ko in range(8):
                        nc.tensor.matmul(
                            ps, Wk_t[:, ko, ao * 128:(ao + 1) * 128],
                            iKT_t[:, ko, nt * 512:(nt + 1) * 512],
                            start=(ko == 0), stop=(ko == 7))
                    nc.vector.tensor_copy(kT_t[:, ao, nt * 512:(nt + 1) * 512], ps)
            for mo in range(8):
                ps = psA.tile([128, 512], F32, tag="mm")
                for ko in range(8):
                    nc.tensor.matmul(
                        ps, iKT_t[:, ko, mo * 128:(mo + 1) * 128], Wv_t[:, ko, :],
                        start=(ko == 0), stop=(ko == 7))
                nc.vector.tensor_copy(
                    v_t[:, mo, :, 0:64], ps.rearrange("p (h a) -> p h a", a=64))
            nc.vector.tensor_copy(
                v_t[:, :, :, 64], ones_f32.rearrange("p (a b) -> p a b", a=8))

            # ---- phase C: qT from iQT
            iQT_t = big_in.tile([128, 8, NQ], F32R, tag="bigin")
            nc.sync.dma_start(
                out=iQT_t, in_=iQT_h.ap().rearrange("(o p) q -> p o q", p=128))
            Wq_t = wpool.tile([128, 8, HS], F32R, tag="w")
            nc.sync.dma_start(
                out=Wq_t, in_=Wq_h.ap().rearrange("(o p) a -> p o a", p=128))
            qT_t = actpool.tile([128, 4, NQ], F32R)
            for ao in range(4):
                for nt in range(2):
                    ps = psA.tile([128, 512], F32, tag="mm")
                    for ko in range(8):
                        nc.tensor.matmul(
                            ps, Wq_t[:, ko, ao * 128:(ao + 1) * 128],
                            iQT_t[:, ko, nt * 512:(nt + 1) * 512],
                            start=(ko == 0), stop=(ko == 7))
                    nc.vector.tensor_copy(qT_t[:, ao, nt * 512:(nt + 1) * 512], ps)

            # Wo loaded up-front (reuses a w slot after Wk's last read)
            Wo_t = wpool.tile([128, 4, D], F32R, tag="w")
            nc.sync.dma_start(
                out=Wo_t, in_=Wo_h.ap().rearrange("(o p) d -> p o d", p=128))

            oT_t = actpool.tile([128, 4, NQ], F32R)       # packed normalized oT

            # ---- phase D: per (head, q-half)
            for h in range(HPC):
                pb = 64 * (h % 2)
                ho = h // 2
                for qh in range(2):
                    q0 = qh * 512
                    eT_t = epool.tile([128, 8, 512], F32R, tag="eT")
                    for so in range(8):
                        ps = psA.tile([128, 512], F32, tag="mm")
                        nc.tensor.matmul(
                            ps, ident_bf, mnT_t[:, so, q0:q0 + 512],
                            start=True, stop=False)
                        nc.tensor.matmul(
                            ps, kT_t[pb:pb + 64, ho, so * 128:(so + 1) * 128],
                            qT_t[pb:pb + 64, ho, q0:q0 + 512],
                            start=False, stop=True)
                        nc.scalar.activation(
                            eT_t[:, so, :], ps,
                            mybir.ActivationFunctionType.Exp, scale=0.125)
                    # AV (+denominator in row 64)
                    po = psB.tile([65, 512], F32, tag="oT")
                    for so in range(8):
                        nc.tensor.matmul(
                            po, v_t[:, so, h, :], eT_t[:, so, :],
                            start=(so == 0), stop=(so == 7))
                    # reciprocal of denominators (row 64, [1,512] in q)
                    rt = small.tile([128, 512], F32R, tag="rt")
                    with nc.allow_low_precision(reason="f32r is 4-byte fp32 storage"):
                        nc.vector.reciprocal(rt[64:65, :], po[64:65, :])
                    # per-partition layout of recip for attn evict: rc[p, j]=r[j*128+p]
                    # (partition-crossing move -> bounce through DRAM scratch)
                    scr = dpool.tile([512], F32R, tag="scr")
                    nc.sync.dma_start(out=scr, in_=rt[64:65, :])
                    rc = small.tile([128, 4], F32R, tag="rc")
                    nc.sync.dma_start(
                        out=rc, in_=scr.rearrange("(j p) -> p j", p=128))
                    # broadcast recip along a for oT normalize
                    bc = psA.tile([128, 512], F32, tag="mm")
                    nc.tensor.matmul(
                        bc[0:64, :], ones_t[64:65, :], rt[64:65, :],
                        start=True, stop=True)
                    bc_s = small.tile([64, 512], F32R, tag="bc")
                    nc.scalar.copy(bc_s, bc[0:64, :])
                    oT_ev = small.tile([64, 512], F32R, tag="oT_ev")
                    nc.vector.tensor_mul(oT_ev, po[0:64, :], bc_s)
                    nc.sync.dma_start(
                        out=oT_t[pb:pb + 64, ho, q0:q0 + 512], in_=oT_ev)
                    # attn output: PE-transpose eT -> [q,s], normalize on evict
                    for qc in range(4):
                        for sh in range(2):
                            pt = psA.tile([128, 512], F32R, tag="mm")
                            for j in range(4):
                                nc.tensor.transpose(
                                    pt[:, j * 128:(j + 1) * 128],
                                    eT_t[:, sh * 4 + j, qc * 128:(qc + 1) * 128],
                                    ident_t)
                            at = stage.tile([128, 512], F32, tag="attn")
                            rc_f32 = rc[:, qc:qc + 1].bitcast(F32)
                            if (qc + sh) % 2 == 0:
                                nc.vector.tensor_scalar_mul(at, pt, rc_f32)
                            else:
                                nc.scalar.mul(at, pt, rc_f32)
                            nc.sync.dma_start(
                                out=attn_h.ap()[h, q0 + qc * 128:q0 + (qc + 1) * 128,
                                                sh * 512:(sh + 1) * 512],
                                in_=at)

            # ---- phase E: out_p = oT^T @ Wo
            for qt in range(8):
                for nt in range(2):
                    ps = psA.tile([128, 512], F32, tag="mm")
                    for ko in range(4):
                        nc.tensor.matmul(
                            ps, oT_t[:, ko, qt * 128:(qt + 1) * 128],
                            Wo_t[:, ko, nt * 512:(nt + 1) * 512],
                            start=(ko == 0), stop=(ko == 3))
                    ot = stage.tile([128, 512], F32, tag="out", bufs=2)
                    nc.vector.tensor_copy(ot, ps)
                    nc.sync.dma_start(
                        out=outp_h.ap()[qt * 128:(qt + 1) * 128,
                                        nt * 512:(nt + 1) * 512],
                        in_=ot)

    nc.finalize()
    return nc


@functools.lru_cache(maxsize=1)
def _get_program():
    return _build_program()


def _shard_inputs(iQ, iK, mask, Wq, Wkv, Wo):
    iQ = np.asarray(iQ, dtype=np.float32)
    iK = np.asarray(iK, dtype=np.float32)
    mask = np.asarray(mask)
    Wq = np.asarray(Wq, dtype=np.float32)
    Wkv = np.asarray(Wkv, dtype=np.float32).reshape(D, 2, H, A)
    Wo = np.asarray(Wo, dtype=np.float32)
    ident = np.eye(128, dtype=np.float32)

    iQT = [np.ascontiguousarray(iQ[b].T) for b in range(B)]
    iKT = [np.ascontiguousarray(iK[b].T) for b in range(B)]
    import ml_dtypes
    mnT = [np.ascontiguousarray(
        (mask[b].T.astype(np.float32) * MASK_NEG).astype(ml_dtypes.bfloat16))
        for b in range(B)]

    in_maps = []
    for c in range(N_CORES):
        b, hh = c // 2, c % 2
        h0 = hh * HPC
        in_maps.append({
            "iQT": iQT[b],
            "iKT": iKT[b],
            "mnT": mnT[b],
            "Wq_s": np.ascontiguousarray(
                Wq.reshape(D, H, A)[:, h0:h0 + HPC].reshape(D, HS)),
            "Wk_s": np.ascontiguousarray(
                Wkv[:, 0, h0:h0 + HPC].reshape(D, HS)),
            "Wv_s": np.ascontiguousarray(
                Wkv[:, 1, h0:h0 + HPC].reshape(D, HS)),
            "Wo_s": np.ascontiguousarray(Wo[h0 * A:(h0 + HPC) * A, :]),
            "ident": ident,
        })
    return in_maps


def kernel(iQ, iK, mask, Wq, Wkv, Wo):
    nc = _get_program()
    in_maps = _shard_inputs(iQ, iK, mask, Wq, Wkv, Wo)
    res = run_bass_kernel_spmd(nc, in_maps, core_ids=list(range(N_CORES)))
    out = np.zeros((B, NQ, D), dtype=np.float32)
    attn = np.empty((B, H, NQ, SL), dtype=np.float32)
    for c in range(N_CORES):
        b, hh = c // 2, c % 2
        out[b] += res.results[c]["out_p"]
        attn[b, hh * HPC:(hh + 1) * HPC] = res.results[c]["attn_s"]
    return out, attn


# revision 13
# speedup vs baseline: 1.1173x; 1.1173x over previous
"""Cross-attention Trainium2 kernel (8-core SPMD, no collectives).

Problem: B=4, NQ=SL=D=1024, H=16, A=64.
  q = iQ @ Wq; k,v = iK @ Wkv; scores = q k^T / sqrt(A) masked; attn = softmax;
  out = (attn v) @ Wo.  Returns (out, attn).

Sharding: core c -> batch b=c//2, head-half hh=c%2 (8 heads each).
Host pre-transposes iQ/iK/mask per batch (pure data layout); the two
partial out projections per batch are summed on host.

Per-core dataflow (proj matmuls in float32r; eT/v in fp16):
  qT[a,q] = Wq_s^T iQ^T      kT[a,s] = Wk_s^T iK^T      v[s,a] = iK Wv_s
  scoresT[s,q] psum = maskT-copy (split identity, packed) + kT_h^T qT_h
      (head pairs packed on PE rows 0-63 / 64-127)
  eT = Exp(0.125 * psum)  fp16          (ACT, masked entries -> 0)
  oT_ext[a+1,q] += v_ext[s,a+1]^T eT    (ones column -> softmax denoms)
  attn[q,s] = PE-transpose(eT) * recip(denom)   (normalize on PSUM evict)
  out[q,D] = oT_norm^T Wo_s
"""
import functools
import numpy as np

import concourse.bass as bass
import concourse.mybir as mybir
import concourse.tile as tile
from concourse import bacc
from concourse.bass_utils import run_bass_kernel_spmd

B, NQ, SL, D = 4, 1024, 1024, 1024
H, A = 16, 64
HPC = 8            # heads per core
HS = HPC * A       # 512 = per-core slice of hidden
N_CORES = 8
F32 = mybir.dt.float32
F32R = mybir.dt.float32r
BF16 = mybir.dt.bfloat16
F16 = mybir.dt.float16
MASK_NEG = -1e30


def _build_program():
    nc = bacc.Bacc("TRN2", target_bir_lowering=False, debug=False)

    iQT_h = nc.dram_tensor("iQT", [D, NQ], F32R, kind="ExternalInput")
    iKT_h = nc.dram_tensor("iKT", [D, SL], F32R, kind="ExternalInput")
    mnT_h = nc.dram_tensor("mnT", [SL, NQ], BF16, kind="ExternalInput")
    Wq_h = nc.dram_tensor("Wq_s", [D, HS], F32R, kind="ExternalInput")
    Wk_h = nc.dram_tensor("Wk_s", [D, HS], F32R, kind="ExternalInput")
    Wv_h = nc.dram_tensor("Wv_s", [D, HS], F32R, kind="ExternalInput")
    Wo_h = nc.dram_tensor("Wo_s", [HS, D], F32R, kind="ExternalInput")
    ident_h = nc.dram_tensor("ident", [128, 128], F32R, kind="ExternalInput")
    attn_h = nc.dram_tensor("attn_s", [HPC, NQ, SL], F32, kind="ExternalOutput")
    outp_h = nc.dram_tensor("out_p", [NQ, D], F32, kind="ExternalOutput")

    with tile.TileContext(nc) as tc:
        with (
            tc.tile_pool(name="mpool", bufs=1) as mpool,       # maskT resident
            tc.tile_pool(name="actpool", bufs=1) as actpool,   # kT,qT,v_ext,oT resident
            tc.tile_pool(name="stage", bufs=4) as stage,       # attn/out staging
            tc.tile_pool(name="small", bufs=2) as small,
            tc.tile_pool(name="const", bufs=1) as cpool,
            tc.tile_pool(name="dpool", bufs=4, space="DRAM") as dpool,
            tc.tile_pool(name="psA", bufs=6, space="PSUM") as psA,
            tc.tile_pool(name="psB", bufs=2, space="PSUM") as psB,
        ):
            # ---- constants
            ident_t = cpool.tile([128, 128], F32R)
            nc.sync.dma_start(out=ident_t, in_=ident_h.ap())
            ones_f32 = cpool.tile([128, 64], F32)
            nc.vector.memset(ones_f32, 1.0)
            ones_t = cpool.tile([128, 64], F32R)
            nc.vector.tensor_copy(ones_t, ones_f32)
            ident_bf = cpool.tile([128, 128], BF16)
            nc.vector.tensor_copy(ident_bf, ident_t)
            ident_f16 = cpool.tile([128, 128], F16)
            nc.vector.tensor_copy(ident_f16, ident_t)

            # ---- resident mask (transposed, pre-scaled by host)
            mnT_t = mpool.tile([128, 8, NQ], BF16)
            nc.sync.dma_start(
                out=mnT_t, in_=mnT_h.ap().rearrange("(o p) q -> p o q", p=128))

            kT_t = actpool.tile([128, 4, SL], F32R)       # [(2h,a), ho, s]
            v_t = actpool.tile([128, 8, HPC, 65], F16)    # [s_in, so, h, a+ones]
            qT_t = actpool.tile([128, 4, NQ], F32R)
            oT_t = actpool.tile([128, 4, NQ], F32R)       # packed normalized oT

            # ---- phases B/C: projections (input pools released afterwards)
            with (
                tc.tile_pool(name="big_in", bufs=1) as big_in,
                tc.tile_pool(name="wproj", bufs=2) as wproj,
            ):
                iKT_t = big_in.tile([128, 8, SL], F32R, tag="bigin")
                nc.sync.dma_start(
                    out=iKT_t, in_=iKT_h.ap().rearrange("(o p) s -> p o s", p=128))
                Wk_t = wproj.tile([128, 8, HS], F32R, tag="w")
                nc.sync.dma_start(
                    out=Wk_t, in_=Wk_h.ap().rearrange("(o p) a -> p o a", p=128))
                Wv_t = wproj.tile([128, 8, HS], F32R, tag="w")
                nc.sync.dma_start(
                    out=Wv_t, in_=Wv_h.ap().rearrange("(o p) a -> p o a", p=128))

                for ao in range(4):
                    for nt in range(2):
                        ps = psA.tile([128, 512], F32, tag="mm")
                        for ko in range(8):
                            nc.tensor.matmul(
                                ps, Wk_t[:, ko, ao * 128:(ao + 1) * 128],
                                iKT_t[:, ko, nt * 512:(nt + 1) * 512],
                                start=(ko == 0), stop=(ko == 7))
                        nc.vector.tensor_copy(
                            kT_t[:, ao, nt * 512:(nt + 1) * 512], ps)
                for mo in range(8):
                    ps = psA.tile([128, 512], F32, tag="mm")
                    for ko in range(8):
                        nc.tensor.matmul(
                            ps, iKT_t[:, ko, mo * 128:(mo + 1) * 128], Wv_t[:, ko, :],
                            start=(ko == 0), stop=(ko == 7))
                    nc.vector.tensor_copy(
                        v_t[:, mo, :, 0:64], ps.rearrange("p (h a) -> p h a", a=64))
                nc.vector.tensor_copy(
                    v_t[:, :, :, 64], ones_f32.rearrange("p (a b) -> p a b", a=8))

                iQT_t = big_in.tile([128, 8, NQ], F32R, tag="bigin")
                nc.sync.dma_start(
                    out=iQT_t, in_=iQT_h.ap().rearrange("(o p) q -> p o q", p=128))
                Wq_t = wproj.tile([128, 8, HS], F32R, tag="w")
                nc.sync.dma_start(
                    out=Wq_t, in_=Wq_h.ap().rearrange("(o p) a -> p o a", p=128))
                for ao in range(4):
                    for nt in range(2):
                        ps = psA.tile([128, 512], F32, tag="mm")
                        for ko in range(8):
                            nc.tensor.matmul(
                                ps, Wq_t[:, ko, ao * 128:(ao + 1) * 128],
                                iQT_t[:, ko, nt * 512:(nt + 1) * 512],
                                start=(ko == 0), stop=(ko == 7))
                        nc.vector.tensor_copy(
                            qT_t[:, ao, nt * 512:(nt + 1) * 512], ps)

            with (
                tc.tile_pool(name="wo_pool", bufs=1) as wo_pool,
                tc.tile_pool(name="epool", bufs=2) as epool,
            ):
                Wo_t = wo_pool.tile([128, 4, D], F32R)
                nc.sync.dma_start(
                    out=Wo_t, in_=Wo_h.ap().rearrange("(o p) d -> p o d", p=128))

                # ---- phase D: head pairs (2ho, 2ho+1) x q-halves
                for ho in range(4):
                    for qh in range(2):
                        q0 = qh * 512
                        eTs = [epool.tile([128, 8, 512], F16, tag="eT0",
                                          name="eT0"),
                               epool.tile([128, 8, 512], F16, tag="eT1",
                                          name="eT1")]
                        for so in range(8):
                            pss = [psA.tile([128, 512], F32, tag="mm", name="ps0"),
                                   psA.tile([128, 512], F32, tag="mm", name="ps1")]
                            # masked-score PSUMs: split-identity mask copies
                            # (PE cells (r0-63,c0-63)+(r64-127,c64-127) pack)
                            for ps in pss:
                                nc.tensor.matmul(
                                    ps[0:64, :], ident_bf[0:64, 0:64],
                                    mnT_t[0:64, so, q0:q0 + 512],
                                    start=True, stop=False,
                                    tile_position=(0, 0))
                                nc.tensor.matmul(
                                    ps[64:128, :], ident_bf[64:128, 64:128],
                                    mnT_t[64:128, so, q0:q0 + 512],
                                    start=True, stop=False,
                                    tile_position=(64, 64))
                            # QK for the head pair: packed on rows 0-63/64-127
                            for i in range(2):
                                pb = 64 * i
                                nc.tensor.matmul(
                                    pss[i], kT_t[pb:pb + 64, ho,
                                                 so * 128:(so + 1) * 128],
                                    qT_t[pb:pb + 64, ho, q0:q0 + 512],
                                    start=False, stop=True,
                                    tile_position=(pb, 0))
                            for i in range(2):
                                nc.scalar.activation(
                                    eTs[i][:, so, :], pss[i],
                                    mybir.ActivationFunctionType.Exp, scale=0.125)
                        for i in range(2):
                            h = 2 * ho + i
                            pb = 64 * i
                            eT_t = eTs[i]
                            # AV (+denominator in row 64)
                            po = psB.tile([65, 512], F32, tag="oT")
                            for so in range(8):
                                nc.tensor.matmul(
                                    po, v_t[:, so, h, :], eT_t[:, so, :],
                                    start=(so == 0), stop=(so == 7))
                            rt = small.tile([128, 512], F32R, tag="rt")
                            with nc.allow_low_precision(reason="f32r=4B fp32"):
                                nc.vector.reciprocal(rt[64:65, :], po[64:65, :])
                            # recip per-partition layout via DRAM bounce
                            scr = dpool.tile([512], F32R, tag="scr")
                            nc.sync.dma_start(out=scr, in_=rt[64:65, :])
                            rc = small.tile([128, 4], F32R, tag="rc")
                            nc.sync.dma_start(
                                out=rc, in_=scr.rearrange("(j p) -> p j", p=128))
                            # broadcast recip along a for oT normalize
                            bc = psA.tile([128, 512], F32, tag="mm")
                            nc.tensor.matmul(
                                bc[0:64, :], ones_t[64:65, :], rt[64:65, :],
                                start=True, stop=True)
                            bc_s = small.tile([64, 512], F32R, tag="bc")
                            nc.scalar.copy(bc_s, bc[0:64, :])
                            oT_ev = small.tile([64, 512], F32R, tag="oT_ev")
                            nc.vector.tensor_mul(oT_ev, po[0:64, :], bc_s)
                            nc.sync.dma_start(
                                out=oT_t[pb:pb + 64, ho, q0:q0 + 512], in_=oT_ev)
                            # attn: PE-transpose eT -> [q,s], normalize on evict
                            for qc in range(4):
                                for sh in range(2):
                                    pt = psA.tile([128, 512], F16, tag="mm")
                                    for j in range(4):
                                        nc.tensor.transpose(
                                            pt[:, j * 128:(j + 1) * 128],
                                            eT_t[:, sh * 4 + j,
                                                 qc * 128:(qc + 1) * 128],
                                            ident_f16)
                                    at = stage.tile([128, 512], F32, tag="attn")
                                    rc_f32 = rc[:, qc:qc + 1].bitcast(F32)
                                    if (qc + 2 * sh) % 4 == 3:
                                        nc.scalar.mul(at, pt, rc_f32)
                                    else:
                                        nc.vector.tensor_scalar_mul(at, pt, rc_f32)
                                    nc.sync.dma_start(
                                        out=attn_h.ap()[
                                            h, q0 + qc * 128:q0 + (qc + 1) * 128,
                                            sh * 512:(sh + 1) * 512],
                                        in_=at)

                # ---- phase E: out_p = oT^T @ Wo
                for qt in range(8):
                    for nt in range(2):
                        ps = psA.tile([128, 512], F32, tag="mm")
                        for ko in range(4):
                            nc.tensor.matmul(
                                ps, oT_t[:, ko, qt * 128:(qt + 1) * 128],
                                Wo_t[:, ko, nt * 512:(nt + 1) * 512],
                                start=(ko == 0), stop=(ko == 3))
                        ot = stage.tile([128, 512], F32, tag="out", bufs=2)
                        nc.vector.tensor_copy(ot, ps)
                        nc.sync.dma_start(
                            out=outp_h.ap()[qt * 128:(qt + 1) * 128,
                                            nt * 512:(nt + 1) * 512],
                            in_=ot)

    nc.finalize()
    return nc


@functools.lru_cache(maxsize=1)
def _get_program():
    return _build_program()


def _shard_inputs(iQ, iK, mask, Wq, Wkv, Wo):
    iQ = np.asarray(iQ, dtype=np.float32)
    iK = np.asarray(iK, dtype=np.float32)
    mask = np.asarray(mask)
    Wq = np.asarray(Wq, dtype=np.float32)
    Wkv = np.asarray(Wkv, dtype=np.float32).reshape(D, 2, H, A)
    Wo = np.asarray(Wo, dtype=np.float32)
    ident = np.eye(128, dtype=np.float32)

    iQT = [np.ascontiguousarray(iQ[b].T) for b in range(B)]
    iKT = [np.ascontiguousarray(iK[b].T) for b in range(B)]
    import ml_dtypes
    mnT = [np.ascontiguousarray(
        (mask[b].T.astype(np.float32) * MASK_NEG).astype(ml_dtypes.bfloat16))
        for b in range(B)]

    in_maps = []
    for c in range(N_CORES):
        b, hh = c // 2, c % 2
        h0 = hh * HPC
        in_maps.append({
            "iQT": iQT[b],
            "iKT": iKT[b],
            "mnT": mnT[b],
            "Wq_s": np.ascontiguousarray(
                Wq.reshape(D, H, A)[:, h0:h0 + HPC].reshape(D, HS)),
            "Wk_s": np.ascontiguousarray(
                Wkv[:, 0, h0:h0 + HPC].reshape(D, HS)),
            "Wv_s": np.ascontiguousarray(
                Wkv[:, 1, h0:h0 + HPC].reshape(D, HS)),
            "Wo_s": np.ascontiguousarray(Wo[h0 * A:(h0 + HPC) * A, :]),
            "ident": ident,
        })
    return in_maps


def kernel(iQ, iK, mask, Wq, Wkv, Wo):
    nc = _get_program()
    in_maps = _shard_inputs(iQ, iK, mask, Wq, Wkv, Wo)
    res = run_bass_kernel_spmd(nc, in_maps, core_ids=list(range(N_CORES)))
    out = np.zeros((B, NQ, D), dtype=np.float32)
    attn = np.empty((B, H, NQ, SL), dtype=np.float32)
    for c in range(N_CORES):
        b, hh = c // 2, c % 2
        out[b] += res.results[c]["out_p"]
        attn[b, hh * HPC:(hh + 1) * HPC] = res.results[c]["attn_s"]
    return out, attn


# revision 16
# speedup vs baseline: 1.4077x; 1.2599x over previous
"""Cross-attention Trainium2 kernel (8-core SPMD, no collectives).

Problem: B=4, NQ=SL=D=1024, H=16, A=64.
  q = iQ @ Wq; k,v = iK @ Wkv; scores = q k^T / sqrt(A) masked; attn = softmax;
  out = (attn v) @ Wo.  Returns (out, attn).

Sharding: core c -> batch b=c//2, head-half hh=c%2 (8 heads each).
Host pre-transposes iQ/iK/mask per batch (pure data layout); the two
partial out projections per batch are summed on host.

Per-core dataflow (proj matmuls in float32r; eT/v in fp16):
  qT[a,q] = Wq_s^T iQ^T      kT[a,s] = Wk_s^T iK^T      v[s,a] = iK Wv_s
  scoresT[s,q] psum = maskT-copy (split identity, packed) + kT_h^T qT_h
      (head pairs packed on PE rows 0-63 / 64-127)
  eT = Exp(0.125 * psum)  fp16          (ACT, masked entries -> 0)
  oT_ext[a+1,q] += v_ext[s,a+1]^T eT    (ones column -> softmax denoms)
  attn[q,s] = PE-transpose(eT) * recip(denom)   (normalize on PSUM evict)
  out[q,D] = oT_norm^T Wo_s
"""
import functools
import numpy as np

import concourse.bass as bass
import concourse.mybir as mybir
import concourse.tile as tile
from concourse import bacc
from concourse.bass_utils import run_bass_kernel_spmd

B, NQ, SL, D = 4, 1024, 1024, 1024
H, A = 16, 64
HPC = 8            # heads per core
HS = HPC * A       # 512 = per-core slice of hidden
N_CORES = 8
F32 = mybir.dt.float32
F32R = mybir.dt.float32r
BF16 = mybir.dt.bfloat16
F16 = mybir.dt.float16
FP8 = mybir.dt.float8e5
MASK_NEG = -4096.0


def _build_program():
    nc = bacc.Bacc("TRN2", target_bir_lowering=False, debug=False)

    iQT_h = nc.dram_tensor("iQT", [D, NQ], F32R, kind="ExternalInput")
    iKT_h = nc.dram_tensor("iKT", [D, SL], F32R, kind="ExternalInput")
    mnT_h = nc.dram_tensor("mnT", [SL, NQ], FP8, kind="ExternalInput")
    Wq_h = nc.dram_tensor("Wq_s", [D, HS], F32R, kind="ExternalInput")
    Wk_h = nc.dram_tensor("Wk_s", [D, HS], F32R, kind="ExternalInput")
    Wv_h = nc.dram_tensor("Wv_s", [D, HS], F32R, kind="ExternalInput")
    Wo_h = nc.dram_tensor("Wo_s", [HS, D], F32R, kind="ExternalInput")
    ident_h = nc.dram_tensor("ident", [128, 128], F32R, kind="ExternalInput")
    attn_h = nc.dram_tensor("attn_s", [HPC, NQ, SL], F32, kind="ExternalOutput")
    outp_h = nc.dram_tensor("out_p", [NQ, D], F32, kind="ExternalOutput")

    with tile.TileContext(nc) as tc:
        with (
            tc.tile_pool(name="mpool", bufs=1) as mpool,       # maskT resident
            tc.tile_pool(name="actpool", bufs=1) as actpool,   # kT,qT,v_ext,oT resident
            tc.tile_pool(name="stage", bufs=3) as stage,       # attn/out staging
            tc.tile_pool(name="small", bufs=2) as small,
            tc.tile_pool(name="const", bufs=1) as cpool,
            tc.tile_pool(name="dpool", bufs=4, space="DRAM") as dpool,
            tc.tile_pool(name="psA", bufs=6, space="PSUM") as psA,
            tc.tile_pool(name="psB", bufs=2, space="PSUM") as psB,
        ):
            # ---- constants
            ident_t = cpool.tile([128, 128], F32R)
            nc.sync.dma_start(out=ident_t, in_=ident_h.ap())
            ones_f32 = cpool.tile([128, 64], F32)
            nc.vector.memset(ones_f32, 1.0)
            ones_t = cpool.tile([128, 64], F32R)
            nc.vector.tensor_copy(ones_t, ones_f32)
            ident_bf = cpool.tile([128, 128], FP8)
            nc.vector.tensor_copy(ident_bf, ident_t)
            ident_f16 = cpool.tile([128, 128], F16)
            nc.vector.tensor_copy(ident_f16, ident_t)

            mnT_t = mpool.tile([128, 8, NQ], FP8)
            kT_t = actpool.tile([128, 4, SL], F32R)       # [(2h,a), ho, s]
            v_t = actpool.tile([128, 8, HPC, 65], F16)    # [s_in, so, h, a+ones]
            qT_t = actpool.tile([128, 4, NQ], F32R)
            oT_t = actpool.tile([128, 4, NQ], F32R)       # packed normalized oT

            # ---- phases B/C: projections (input pools released afterwards)
            with (
                tc.tile_pool(name="big_in", bufs=2) as big_in,
                tc.tile_pool(name="wproj", bufs=2) as wproj,
            ):
                iKT_t = big_in.tile([128, 8, SL], F32R, tag="bigin")
                Wk_t = wproj.tile([128, 8, HS], F32R, tag="w")
                Wv_t = wproj.tile([128, 8, HS], F32R, tag="w")
                iKT_r = iKT_h.ap().rearrange("(o p) s -> p o s", p=128)
                Wk_r = Wk_h.ap().rearrange("(o p) a -> p o a", p=128)
                Wv_r = Wv_h.ap().rearrange("(o p) a -> p o a", p=128)
                for ko in range(8):
                    nc.sync.dma_start(out=iKT_t[:, ko, :], in_=iKT_r[:, ko, :])
                    nc.sync.dma_start(out=Wk_t[:, ko, :], in_=Wk_r[:, ko, :])
                for ko in range(8):
                    nc.sync.dma_start(out=Wv_t[:, ko, :], in_=Wv_r[:, ko, :])

                for ao in range(4):
                    for nt in range(2):
                        ps = psA.tile([128, 512], F32, tag="mm")
                        for ko in range(8):
                            nc.tensor.matmul(
                                ps, Wk_t[:, ko, ao * 128:(ao + 1) * 128],
                                iKT_t[:, ko, nt * 512:(nt + 1) * 512],
                                start=(ko == 0), stop=(ko == 7))
                        nc.vector.tensor_copy(
                            kT_t[:, ao, nt * 512:(nt + 1) * 512], ps)
                for mo in range(8):
                    ps = psA.tile([128, 512], F32, tag="mm")
                    for ko in range(8):
                        nc.tensor.matmul(
                            ps, iKT_t[:, ko, mo * 128:(mo + 1) * 128], Wv_t[:, ko, :],
                            start=(ko == 0), stop=(ko == 7))
                    nc.vector.tensor_copy(
                        v_t[:, mo, :, 0:64], ps.rearrange("p (h a) -> p h a", a=64))
                nc.vector.tensor_copy(
                    v_t[:, :, :, 64], ones_f32.rearrange("p (a b) -> p a b", a=8))

                # mask loads (needed from phase D on)
                mnT_r = mnT_h.ap().rearrange("(o p) q -> p o q", p=128)
                for so in range(8):
                    nc.sync.dma_start(out=mnT_t[:, so, :], in_=mnT_r[:, so, :])

                iQT_t = big_in.tile([128, 8, NQ], F32R, tag="bigin")
                Wq_t = wproj.tile([128, 8, HS], F32R, tag="w")
                iQT_r = iQT_h.ap().rearrange("(o p) q -> p o q", p=128)
                Wq_r = Wq_h.ap().rearrange("(o p) a -> p o a", p=128)
                for ko in range(8):
                    nc.sync.dma_start(out=iQT_t[:, ko, :], in_=iQT_r[:, ko, :])
                    nc.sync.dma_start(out=Wq_t[:, ko, :], in_=Wq_r[:, ko, :])
                for ao in range(4):
                    for nt in range(2):
                        ps = psA.tile([128, 512], F32, tag="mm")
                        for ko in range(8):
                            nc.tensor.matmul(
                                ps, Wq_t[:, ko, ao * 128:(ao + 1) * 128],
                                iQT_t[:, ko, nt * 512:(nt + 1) * 512],
                                start=(ko == 0), stop=(ko == 7))
                        nc.vector.tensor_copy(
                            qT_t[:, ao, nt * 512:(nt + 1) * 512], ps)

            with (
                tc.tile_pool(name="wo_pool", bufs=1) as wo_pool,
                tc.tile_pool(name="epool", bufs=2) as epool,
            ):
                Wo_t = wo_pool.tile([128, 4, D], F32R)
                nc.sync.dma_start(
                    out=Wo_t, in_=Wo_h.ap().rearrange("(o p) d -> p o d", p=128))

                # ---- phase D: head pairs (2ho, 2ho+1) x q-halves,
                # software-pipelined: iteration N's attn transposes/evicts are
                # emitted during iteration N+1 (recip chain is ready by then).
                def emit_scores(ho, qh):
                    q0 = qh * 512
                    eTs = [[epool.tile([128, 512], F16, tag=f"eT{i}_{so}",
                                       name=f"eT{i}_{so}")
                            for so in range(8)] for i in range(2)]
                    for so in range(8):
                        pss = [psA.tile([128, 512], F32, tag="mm", name="ps0"),
                               psA.tile([128, 512], F32, tag="mm", name="ps1")]
                        for ps in pss:
                            nc.tensor.matmul(
                                ps[0:64, :], ident_bf[0:64, 0:64],
                                mnT_t[0:64, so, q0:q0 + 512],
                                start=True, stop=False, tile_position=(0, 0))
                            nc.tensor.matmul(
                                ps[64:128, :], ident_bf[64:128, 64:128],
                                mnT_t[64:128, so, q0:q0 + 512],
                                start=True, stop=False, tile_position=(64, 64))
                        for i in range(2):
                            pb = 64 * i
                            nc.tensor.matmul(
                                pss[i], kT_t[pb:pb + 64, ho,
                                             so * 128:(so + 1) * 128],
                                qT_t[pb:pb + 64, ho, q0:q0 + 512],
                                start=False, stop=True, tile_position=(pb, 0))
                        for i in range(2):
                            nc.scalar.activation(
                                eTs[i][so], pss[i],
                                mybir.ActivationFunctionType.Exp, scale=0.125)
                    return eTs

                def emit_av(ho, qh, eTs):
                    q0 = qh * 512
                    rcs = []
                    for i in range(2):
                        h = 2 * ho + i
                        eT_t = eTs[i]
                        po = psB.tile([65, 512], F32, tag="oT", name="po")
                        for so in range(8):
                            nc.tensor.matmul(
                                po, v_t[:, so, h, :], eT_t[so],
                                start=(so == 0), stop=(so == 7))
                        # quick-evict po so the PSUM bank frees immediately
                        po_s = small.tile([65, 512], F32R, tag="po_s",
                                          name="po_s")
                        nc.vector.tensor_copy(po_s, po)
                        rt = small.tile([128, 512], F32R, tag="rt", name="rt")
                        with nc.allow_low_precision(reason="f32r=4B fp32"):
                            nc.vector.reciprocal(rt[64:65, :], po_s[64:65, :])
                        # recip per-partition layout via DRAM bounce
                        scr = dpool.tile([512], F32R, tag="scr", name="scr")
                        nc.sync.dma_start(out=scr, in_=rt[64:65, :])
                        rc = small.tile([128, 4], F32R, tag="rc", name="rc",
                                        bufs=4)
                        nc.sync.dma_start(
                            out=rc, in_=scr.rearrange("(j p) -> p j", p=128))
                        rcs.append(rc)
                        # broadcast recip along a for oT normalize
                        bc = psA.tile([128, 512], F32, tag="mm", name="bc")
                        nc.tensor.matmul(
                            bc[0:64, :], ones_t[64:65, :], rt[64:65, :],
                            start=True, stop=True)
                        oT_ev = small.tile([64, 512], F32R, tag="oT_ev",
                                           name="oT_ev")
                        nc.vector.tensor_mul(oT_ev, bc[0:64, :], po_s[0:64, :])
                        pb = 64 * i
                        nc.sync.dma_start(
                            out=oT_t[pb:pb + 64, ho, q0:q0 + 512], in_=oT_ev)
                    return rcs

                def emit_attn_out(ho, qh, eTs, rcs):
                    q0 = qh * 512
                    for i in range(2):
                        h = 2 * ho + i
                        eT_t = eTs[i]
                        rc = rcs[i]
                        for qc in range(4):
                            for sh in range(2):
                                pt = psA.tile([128, 512], F16, tag="mm",
                                              name="pt")
                                for j in range(4):
                                    nc.tensor.transpose(
                                        pt[:, j * 128:(j + 1) * 128],
                                        eT_t[sh * 4 + j][:,
                                             qc * 128:(qc + 1) * 128],
                                        ident_f16)
                                at = stage.tile([128, 512], F32, tag="attn",
                                                name="at")
                                rc_f32 = rc[:, qc:qc + 1].bitcast(F32)
                                if (qc + 2 * sh) % 4 == 3:
                                    nc.scalar.mul(at, pt, rc_f32)
                                else:
                                    nc.vector.tensor_scalar_mul(at, pt, rc_f32)
                                nc.sync.dma_start(
                                    out=attn_h.ap()[
                                        h, q0 + qc * 128:q0 + (qc + 1) * 128,
                                        sh * 512:(sh + 1) * 512],
                                    in_=at)

                prev = None
                for ho in range(4):
                    for qh in range(2):
                        eTs = emit_scores(ho, qh)
                        if prev is not None:
                            emit_attn_out(*prev)
                        rcs = emit_av(ho, qh, eTs)
                        prev = (ho, qh, eTs, rcs)
                emit_attn_out(*prev)

                # ---- phase E: out_p = oT^T @ Wo
                for qt in range(8):
                    for nt in range(2):
                        ps = psA.tile([128, 512], F32, tag="mm")
                        for ko in range(4):
                            nc.tensor.matmul(
                                ps, oT_t[:, ko, qt * 128:(qt + 1) * 128],
                                Wo_t[:, ko, nt * 512:(nt + 1) * 512],
                                start=(ko == 0), stop=(ko == 3))
                        ot = stage.tile([128, 512], F32, tag="out", bufs=2)
                        nc.vector.tensor_copy(ot, ps)
                        nc.sync.dma_start(
                            out=outp_h.ap()[qt * 128:(qt + 1) * 128,
                                            nt * 512:(nt + 1) * 512],
                            in_=ot)

    nc.finalize()
    return nc


@functools.lru_cache(maxsize=1)
def _get_program():
    return _build_program()


def _shard_inputs(iQ, iK, mask, Wq, Wkv, Wo):
    iQ = np.asarray(iQ, dtype=np.float32)
    iK = np.asarray(iK, dtype=np.float32)
    mask = np.asarray(mask)
    Wq = np.asarray(Wq, dtype=np.float32)
    Wkv = np.asarray(Wkv, dtype=np.float32).reshape(D, 2, H, A)
    Wo = np.asarray(Wo, dtype=np.float32)
    ident = np.eye(128, dtype=np.float32)

    iQT = [np.ascontiguousarray(iQ[b].T) for b in range(B)]
    iKT = [np.ascontiguousarray(iK[b].T) for b in range(B)]
    import ml_dtypes
    mnT = [np.ascontiguousarray(
        (mask[b].T.astype(np.float32) * MASK_NEG).astype(ml_dtypes.float8_e5m2))
        for b in range(B)]

    in_maps = []
    for c in range(N_CORES):
        b, hh = c // 2, c % 2
        h0 = hh * HPC
        in_maps.append({
            "iQT": iQT[b],
            "iKT": iKT[b],
            "mnT": mnT[b],
            "Wq_s": np.ascontiguousarray(
                Wq.reshape(D, H, A)[:, h0:h0 + HPC].reshape(D, HS)),
            "Wk_s": np.ascontiguousarray(
                Wkv[:, 0, h0:h0 + HPC].reshape(D, HS)),
            "Wv_s": np.ascontiguousarray(
                Wkv[:, 1, h0:h0 + HPC].reshape(D, HS)),
            "Wo_s": np.ascontiguousarray(Wo[h0 * A:(h0 + HPC) * A, :]),
            "ident": ident,
        })
    return in_maps


def kernel(iQ, iK, mask, Wq, Wkv, Wo):
    nc = _get_program()
    in_maps = _shard_inputs(iQ, iK, mask, Wq, Wkv, Wo)
    res = run_bass_kernel_spmd(nc, in_maps, core_ids=list(range(N_CORES)))
    out = np.zeros((B, NQ, D), dtype=np.float32)
    attn = np.empty((B, H, NQ, SL), dtype=np.float32)
    for c in range(N_CORES):
        b, hh = c // 2, c % 2
        out[b] += res.results[c]["out_p"]
        attn[b, hh * HPC:(hh + 1) * HPC] = res.results[c]["attn_s"]
    return out, attn


# revision 17
# speedup vs baseline: 1.4943x; 1.0615x over previous
"""Cross-attention Trainium2 kernel (8-core SPMD, no collectives).

Problem: B=4, NQ=SL=D=1024, H=16, A=64.
  q = iQ @ Wq; k,v = iK @ Wkv; scores = q k^T / sqrt(A) masked; attn = softmax;
  out = (attn v) @ Wo.  Returns (out, attn).

Sharding: core c -> batch b=c//2, head-half hh=c%2 (8 heads each).
Host pre-transposes iQ/iK/mask per batch (pure data layout); the two
partial out projections per batch are summed on host.

Per-core dataflow (proj matmuls in float32r; eT/v in fp16):
  qT[a,q] = Wq_s^T iQ^T      kT[a,s] = Wk_s^T iK^T      v[s,a] = iK Wv_s
  scoresT[s,q] psum = maskT-copy (split identity, packed) + kT_h^T qT_h
      (head pairs packed on PE rows 0-63 / 64-127)
  eT = Exp(0.125 * psum)  fp16          (ACT, masked entries -> 0)
  oT_ext[a+1,q] += v_ext[s,a+1]^T eT    (ones column -> softmax denoms)
  attn[q,s] = PE-transpose(eT) * recip(denom)   (normalize on PSUM evict)
  out[q,D] = oT_norm^T Wo_s
"""
import functools
import numpy as np

import concourse.bass as bass
import concourse.mybir as mybir
import concourse.tile as tile
from concourse import bacc
from concourse.bass_utils import run_bass_kernel_spmd

B, NQ, SL, D = 4, 1024, 1024, 1024
H, A = 16, 64
HPC = 8            # heads per core
HS = HPC * A       # 512 = per-core slice of hidden
N_CORES = 8
F32 = mybir.dt.float32
F32R = mybir.dt.float32r
BF16 = mybir.dt.bfloat16
F16 = mybir.dt.float16
FP8 = mybir.dt.float8e5
MASK_NEG = -4096.0


def _build_program():
    nc = bacc.Bacc("TRN2", target_bir_lowering=False, debug=False)

    iQT_h = nc.dram_tensor("iQT", [D, NQ], F32R, kind="ExternalInput")
    iKT_h = nc.dram_tensor("iKT", [D, SL], F32R, kind="ExternalInput")
    mnT_h = nc.dram_tensor("mnT", [SL, NQ], FP8, kind="ExternalInput")
    Wq_h = nc.dram_tensor("Wq_s", [D, HS], F32R, kind="ExternalInput")
    Wk_h = nc.dram_tensor("Wk_s", [D, HS], F32R, kind="ExternalInput")
    Wv_h = nc.dram_tensor("Wv_s", [D, HS], F32R, kind="ExternalInput")
    Wo_h = nc.dram_tensor("Wo_s", [HS, D], F32R, kind="ExternalInput")
    ident_h = nc.dram_tensor("ident", [128, 128], F32R, kind="ExternalInput")
    attn_h = nc.dram_tensor("attn_s", [HPC, NQ, SL], F16, kind="ExternalOutput")
    outp_h = nc.dram_tensor("out_p", [NQ, D], F32, kind="ExternalOutput")

    with tile.TileContext(nc) as tc:
        with (
            tc.tile_pool(name="mpool", bufs=1) as mpool,       # maskT resident
            tc.tile_pool(name="actpool", bufs=1) as actpool,   # kT,qT,v_ext,oT resident
            tc.tile_pool(name="stage", bufs=3) as stage,       # attn/out staging
            tc.tile_pool(name="small", bufs=2) as small,
            tc.tile_pool(name="const", bufs=1) as cpool,
            tc.tile_pool(name="dpool", bufs=4, space="DRAM") as dpool,
            tc.tile_pool(name="psA", bufs=6, space="PSUM") as psA,
            tc.tile_pool(name="psB", bufs=2, space="PSUM") as psB,
        ):
            # ---- constants
            ident_t = cpool.tile([128, 128], F32R)
            nc.sync.dma_start(out=ident_t, in_=ident_h.ap())
            ones_f32 = cpool.tile([128, 64], F32)
            nc.vector.memset(ones_f32, 1.0)
            ones_t = cpool.tile([128, 64], F32R)
            nc.vector.tensor_copy(ones_t, ones_f32)
            ident_bf = cpool.tile([128, 128], FP8)
            nc.vector.tensor_copy(ident_bf, ident_t)
            ident_f16 = cpool.tile([128, 128], F16)
            nc.vector.tensor_copy(ident_f16, ident_t)

            mnT_t = mpool.tile([128, 8, NQ], FP8)
            kT_t = actpool.tile([128, 4, SL], F32R)       # [(2h,a), ho, s]
            v_t = actpool.tile([128, 8, HPC, 65], F16)    # [s_in, so, h, a+ones]
            qT_t = actpool.tile([128, 4, NQ], F32R)
            oT_t = actpool.tile([128, 4, NQ], F32R)       # packed normalized oT

            # ---- phases B/C: projections (input pools released afterwards)
            with (
                tc.tile_pool(name="big_in", bufs=2) as big_in,
                tc.tile_pool(name="wproj", bufs=2) as wproj,
            ):
                iKT_t = big_in.tile([128, 8, SL], F32R, tag="bigin")
                Wk_t = wproj.tile([128, 8, HS], F32R, tag="w")
                Wv_t = wproj.tile([128, 8, HS], F32R, tag="w")
                iKT_r = iKT_h.ap().rearrange("(o p) s -> p o s", p=128)
                Wk_r = Wk_h.ap().rearrange("(o p) a -> p o a", p=128)
                Wv_r = Wv_h.ap().rearrange("(o p) a -> p o a", p=128)
                for ko in range(8):
                    nc.sync.dma_start(out=iKT_t[:, ko, :], in_=iKT_r[:, ko, :])
                    nc.sync.dma_start(out=Wk_t[:, ko, :], in_=Wk_r[:, ko, :])
                for ko in range(8):
                    nc.sync.dma_start(out=Wv_t[:, ko, :], in_=Wv_r[:, ko, :])

                for ao in range(4):
                    for nt in range(2):
                        ps = psA.tile([128, 512], F32, tag="mm")
                        for ko in range(8):
                            nc.tensor.matmul(
                                ps, Wk_t[:, ko, ao * 128:(ao + 1) * 128],
                                iKT_t[:, ko, nt * 512:(nt + 1) * 512],
                                start=(ko == 0), stop=(ko == 7))
                        nc.vector.tensor_copy(
                            kT_t[:, ao, nt * 512:(nt + 1) * 512], ps)
                for mo in range(8):
                    ps = psA.tile([128, 512], F32, tag="mm")
                    for ko in range(8):
                        nc.tensor.matmul(
                            ps, iKT_t[:, ko, mo * 128:(mo + 1) * 128], Wv_t[:, ko, :],
                            start=(ko == 0), stop=(ko == 7))
                    nc.vector.tensor_copy(
                        v_t[:, mo, :, 0:64], ps.rearrange("p (h a) -> p h a", a=64))
                nc.vector.tensor_copy(
                    v_t[:, :, :, 64], ones_f32.rearrange("p (a b) -> p a b", a=8))

                # mask loads (needed from phase D on)
                mnT_r = mnT_h.ap().rearrange("(o p) q -> p o q", p=128)
                for so in range(8):
                    nc.sync.dma_start(out=mnT_t[:, so, :], in_=mnT_r[:, so, :])

                iQT_t = big_in.tile([128, 8, NQ], F32R, tag="bigin")
                Wq_t = wproj.tile([128, 8, HS], F32R, tag="w")
                iQT_r = iQT_h.ap().rearrange("(o p) q -> p o q", p=128)
                Wq_r = Wq_h.ap().rearrange("(o p) a -> p o a", p=128)
                for ko in range(8):
                    nc.sync.dma_start(out=iQT_t[:, ko, :], in_=iQT_r[:, ko, :])
                    nc.sync.dma_start(out=Wq_t[:, ko, :], in_=Wq_r[:, ko, :])
                for ao in range(4):
                    for nt in range(2):
                        ps = psA.tile([128, 512], F32, tag="mm")
                        for ko in range(8):
                            nc.tensor.matmul(
                                ps, Wq_t[:, ko, ao * 128:(ao + 1) * 128],
                                iQT_t[:, ko, nt * 512:(nt + 1) * 512],
                                start=(ko == 0), stop=(ko == 7))
                        nc.vector.tensor_copy(
                            qT_t[:, ao, nt * 512:(nt + 1) * 512], ps)

            with (
                tc.tile_pool(name="wo_pool", bufs=1) as wo_pool,
                tc.tile_pool(name="epool", bufs=2) as epool,
            ):
                Wo_t = wo_pool.tile([128, 4, D], F32R)
                nc.sync.dma_start(
                    out=Wo_t, in_=Wo_h.ap().rearrange("(o p) d -> p o d", p=128))

                # ---- phase D: head pairs (2ho, 2ho+1) x q-halves,
                # software-pipelined: iteration N's attn transposes/evicts are
                # emitted during iteration N+1 (recip chain is ready by then).
                def emit_scores(ho, qh):
                    q0 = qh * 512
                    eTs = [[epool.tile([128, 512], F16, tag=f"eT{i}_{so}",
                                       name=f"eT{i}_{so}")
                            for so in range(8)] for i in range(2)]
                    for so in range(8):
                        pss = [psA.tile([128, 512], F32, tag="mm", name="ps0"),
                               psA.tile([128, 512], F32, tag="mm", name="ps1")]
                        for ps in pss:
                            nc.tensor.matmul(
                                ps[0:64, :], ident_bf[0:64, 0:64],
                                mnT_t[0:64, so, q0:q0 + 512],
                                start=True, stop=False, tile_position=(0, 0))
                            nc.tensor.matmul(
                                ps[64:128, :], ident_bf[64:128, 64:128],
                                mnT_t[64:128, so, q0:q0 + 512],
                                start=True, stop=False, tile_position=(64, 64))
                        for i in range(2):
                            pb = 64 * i
                            nc.tensor.matmul(
                                pss[i], kT_t[pb:pb + 64, ho,
                                             so * 128:(so + 1) * 128],
                                qT_t[pb:pb + 64, ho, q0:q0 + 512],
                                start=False, stop=True, tile_position=(pb, 0))
                        for i in range(2):
                            nc.scalar.activation(
                                eTs[i][so], pss[i],
                                mybir.ActivationFunctionType.Exp, scale=0.125)
                    return eTs

                def emit_av(ho, qh, eTs):
                    q0 = qh * 512
                    rcs = []
                    for i in range(2):
                        h = 2 * ho + i
                        eT_t = eTs[i]
                        po = psB.tile([65, 512], F32, tag="oT", name="po")
                        for so in range(8):
                            nc.tensor.matmul(
                                po, v_t[:, so, h, :], eT_t[so],
                                start=(so == 0), stop=(so == 7))
                        # quick-evict po so the PSUM bank frees immediately
                        po_s = small.tile([65, 512], F32R, tag="po_s",
                                          name="po_s")
                        nc.vector.tensor_copy(po_s, po)
                        rt = small.tile([128, 512], F32R, tag="rt", name="rt")
                        with nc.allow_low_precision(reason="f32r=4B fp32"):
                            nc.vector.reciprocal(rt[64:65, :], po_s[64:65, :])
                        # recip per-partition layout via DRAM bounce
                        scr = dpool.tile([512], F32R, tag="scr", name="scr")
                        nc.sync.dma_start(out=scr, in_=rt[64:65, :])
                        rc = small.tile([128, 4], F32R, tag="rc", name="rc",
                                        bufs=4)
                        nc.sync.dma_start(
                            out=rc, in_=scr.rearrange("(j p) -> p j", p=128))
                        rcs.append(rc)
                        # broadcast recip along a for oT normalize
                        bc = psA.tile([128, 512], F32, tag="mm", name="bc")
                        nc.tensor.matmul(
                            bc[0:64, :], ones_t[64:65, :], rt[64:65, :],
                            start=True, stop=True)
                        oT_ev = small.tile([64, 512], F32R, tag="oT_ev",
                                           name="oT_ev")
                        nc.vector.tensor_mul(oT_ev, bc[0:64, :], po_s[0:64, :])
                        pb = 64 * i
                        nc.sync.dma_start(
                            out=oT_t[pb:pb + 64, ho, q0:q0 + 512], in_=oT_ev)
                    return rcs

                def emit_attn_out(ho, qh, eTs, rcs):
                    q0 = qh * 512
                    for i in range(2):
                        h = 2 * ho + i
                        eT_t = eTs[i]
                        rc = rcs[i]
                        for qc in range(4):
                            for sh in range(2):
                                pt = psA.tile([128, 512], F16, tag="mm",
                                              name="pt")
                                for j in range(4):
                                    nc.tensor.transpose(
                                        pt[:, j * 128:(j + 1) * 128],
                                        eT_t[sh * 4 + j][:,
                                             qc * 128:(qc + 1) * 128],
                                        ident_f16)
                                at = stage.tile([128, 512], F16, tag="attn",
                                                name="at")
                                rc_f32 = rc[:, qc:qc + 1].bitcast(F32)
                                if (qc + 2 * sh) % 4 == 3:
                                    nc.scalar.mul(at, pt, rc_f32)
                                else:
                                    nc.vector.tensor_scalar_mul(at, pt, rc_f32)
                                nc.sync.dma_start(
                                    out=attn_h.ap()[
                                        h, q0 + qc * 128:q0 + (qc + 1) * 128,
                                        sh * 512:(sh + 1) * 512],
                                    in_=at)

                prev = None
                for ho in range(4):
                    for qh in range(2):
                        eTs = emit_scores(ho, qh)
                        if prev is not None:
                            emit_attn_out(*prev)
                        rcs = emit_av(ho, qh, eTs)
                        prev = (ho, qh, eTs, rcs)
                emit_attn_out(*prev)

                # ---- phase E: out_p = oT^T @ Wo
                for qt in range(8):
                    for nt in range(2):
                        ps = psA.tile([128, 512], F32, tag="mm")
                        for ko in range(4):
                            nc.tensor.matmul(
                                ps, oT_t[:, ko, qt * 128:(qt + 1) * 128],
                                Wo_t[:, ko, nt * 512:(nt + 1) * 512],
                                start=(ko == 0), stop=(ko == 3))
                        ot = stage.tile([128, 512], F32, tag="out", bufs=2)
                        nc.vector.tensor_copy(ot, ps)
                        nc.sync.dma_start(
                            out=outp_h.ap()[qt * 128:(qt + 1) * 128,
                                            nt * 512:(nt + 1) * 512],
                            in_=ot)

    nc.finalize()
    return nc


@functools.lru_cache(maxsize=1)
def _get_program():
    return _build_program()


def _shard_inputs(iQ, iK, mask, Wq, Wkv, Wo):
    iQ = np.asarray(iQ, dtype=np.float32)
    iK = np.asarray(iK, dtype=np.float32)
    mask = np.asarray(mask)
    Wq = np.asarray(Wq, dtype=np.float32)
    Wkv = np.asarray(Wkv, dtype=np.float32).reshape(D, 2, H, A)
    Wo = np.asarray(Wo, dtype=np.float32)
    ident = np.eye(128, dtype=np.float32)

    iQT = [np.ascontiguousarray(iQ[b].T) for b in range(B)]
    iKT = [np.ascontiguousarray(iK[b].T) for b in range(B)]
    import ml_dtypes
    mnT = [np.ascontiguousarray(
        (mask[b].T.astype(np.float32) * MASK_NEG).astype(ml_dtypes.float8_e5m2))
        for b in range(B)]

    in_maps = []
    for c in range(N_CORES):
        b, hh = c // 2, c % 2
        h0 = hh * HPC
        in_maps.append({
            "iQT": iQT[b],
            "iKT": iKT[b],
            "mnT": mnT[b],
            "Wq_s": np.ascontiguousarray(
                Wq.reshape(D, H, A)[:, h0:h0 + HPC].reshape(D, HS)),
            "Wk_s": np.ascontiguousarray(
                Wkv[:, 0, h0:h0 + HPC].reshape(D, HS)),
            "Wv_s": np.ascontiguousarray(
                Wkv[:, 1, h0:h0 + HPC].reshape(D, HS)),
            "Wo_s": np.ascontiguousarray(Wo[h0 * A:(h0 + HPC) * A, :]),
            "ident": ident,
        })
    return in_maps


def kernel(iQ, iK, mask, Wq, Wkv, Wo):
    nc = _get_program()
    in_maps = _shard_inputs(iQ, iK, mask, Wq, Wkv, Wo)
    res = run_bass_kernel_spmd(nc, in_maps, core_ids=list(range(N_CORES)))
    out = np.zeros((B, NQ, D), dtype=np.float32)
    attn = np.empty((B, H, NQ, SL), dtype=np.float32)
    for c in range(N_CORES):
        b, hh = c // 2, c % 2
        out[b] += res.results[c]["out_p"]
        attn[b, hh * HPC:(hh + 1) * HPC] = res.results[c]["attn_s"].astype(
            np.float32)
    return out, attn


# revision 22
# speedup vs baseline: 1.6763x; 1.1218x over previous
"""Cross-attention Trainium2 kernel (8-core SPMD, no collectives).

Problem: B=4, NQ=SL=D=1024, H=16, A=64.
  q = iQ @ Wq; k,v = iK @ Wkv; scores = q k^T / sqrt(A) masked; attn = softmax;
  out = (attn v) @ Wo.  Returns (out, attn).

Sharding: core c -> batch b=c//2, head-half hh=c%2 (8 heads each).
Host pre-transposes iQ/iK/mask per batch (pure data layout); the two
partial out projections per batch are summed on host.

Per-core dataflow (proj matmuls in float32r; eT/v in fp16):
  qT[a,q] = Wq_s^T iQ^T      kT[a,s] = Wk_s^T iK^T      v[s,a] = iK Wv_s
  scoresT[s,q] psum = maskT-copy (split identity, packed) + kT_h^T qT_h
      (head pairs packed on PE rows 0-63 / 64-127)
  eT = Exp(0.125 * psum)  fp16          (ACT, masked entries -> 0)
  oT_ext[a+1,q] += v_ext[s,a+1]^T eT    (ones column -> softmax denoms)
  attn[q,s] = PE-transpose(eT) * recip(denom)   (normalize on PSUM evict)
  out[q,D] = oT_norm^T Wo_s
"""
import functools
import numpy as np

import concourse.bass as bass
import concourse.mybir as mybir
import concourse.tile as tile
from concourse import bacc
from concourse.bass_utils import run_bass_kernel_spmd

B, NQ, SL, D = 4, 1024, 1024, 1024
H, A = 16, 64
HPC = 8            # heads per core
HS = HPC * A       # 512 = per-core slice of hidden
N_CORES = 8
F32 = mybir.dt.float32
F32R = mybir.dt.float32r
BF16 = mybir.dt.bfloat16
F16 = mybir.dt.float16
FP8 = mybir.dt.float8e5
MASK_NEG = -4096.0


def _build_program():
    nc = bacc.Bacc("TRN2", target_bir_lowering=False, debug=False)

    iQT_h = nc.dram_tensor("iQT", [D, NQ], F32R, kind="ExternalInput")
    iKT_h = nc.dram_tensor("iKT", [D, SL], F32R, kind="ExternalInput")
    mnT_h = nc.dram_tensor("mnT", [SL, NQ], FP8, kind="ExternalInput")
    Wq_h = nc.dram_tensor("Wq_s", [D, HS], F32R, kind="ExternalInput")
    Wk_h = nc.dram_tensor("Wk_s", [D, HS], F32R, kind="ExternalInput")
    Wv_h = nc.dram_tensor("Wv_s", [D, HS], F32R, kind="ExternalInput")
    Wo_h = nc.dram_tensor("Wo_s", [HS, D], F32R, kind="ExternalInput")
    ident_h = nc.dram_tensor("ident", [128, 128], F32R, kind="ExternalInput")
    attn_h = nc.dram_tensor("attn_s", [HPC, NQ, SL], F16, kind="ExternalOutput")
    outp_h = nc.dram_tensor("out_p", [NQ, D], F32, kind="ExternalOutput")

    with tile.TileContext(nc) as tc:
        with (
            tc.tile_pool(name="mpool", bufs=1) as mpool,       # maskT resident
            tc.tile_pool(name="actpool", bufs=1) as actpool,   # kT,qT,v_ext,oT resident
            tc.tile_pool(name="stage", bufs=6) as stage,       # attn/out staging
            tc.tile_pool(name="small", bufs=3) as small,
            tc.tile_pool(name="const", bufs=1) as cpool,
            tc.tile_pool(name="dpool", bufs=4, space="DRAM") as dpool,
            tc.tile_pool(name="psA", bufs=7, space="PSUM") as psA,
            tc.tile_pool(name="psB", bufs=1, space="PSUM") as psB,
        ):
            # ---- constants
            ident_t = cpool.tile([128, 128], F32R)
            nc.sync.dma_start(out=ident_t, in_=ident_h.ap())
            ones_f32 = cpool.tile([128, 64], F32)
            nc.vector.memset(ones_f32, 1.0)
            ones_t = cpool.tile([128, 64], F32R)
            nc.vector.tensor_copy(ones_t, ones_f32)
            ident_bf = cpool.tile([128, 128], FP8)
            nc.vector.tensor_copy(ident_bf, ident_t)
            ident_f16 = cpool.tile([128, 128], F16)
            nc.vector.tensor_copy(ident_f16, ident_t)

            mnT_t = mpool.tile([128, 8, NQ], FP8)
            kT_t = actpool.tile([128, 4, SL], F32R)       # [(2h,a), ho, s]
            v_t = actpool.tile([128, 8, HPC, 65], F16)    # [s_in, so, h, a+ones]
            qT_t = actpool.tile([128, 4, NQ], F32R)
            oT_t = actpool.tile([128, 4, NQ], F32R)       # packed normalized oT

            # ---- phases B/C: projections (input pools released afterwards)
            with (
                tc.tile_pool(name="big_in", bufs=2) as big_in,
                tc.tile_pool(name="wproj", bufs=2) as wproj,
            ):
                iKT_t = big_in.tile([128, 8, SL], F32R, tag="bigin")
                Wk_t = wproj.tile([128, 8, HS], F32R, tag="w")
                Wv_t = wproj.tile([128, 8, HS], F32R, tag="w")
                iKT_r = iKT_h.ap().rearrange("(o p) s -> p o s", p=128)
                Wk_r = Wk_h.ap().rearrange("(o p) a -> p o a", p=128)
                Wv_r = Wv_h.ap().rearrange("(o p) a -> p o a", p=128)
                for ko in range(8):
                    nc.sync.dma_start(out=iKT_t[:, ko, :], in_=iKT_r[:, ko, :])
                    nc.sync.dma_start(out=Wk_t[:, ko, :], in_=Wk_r[:, ko, :])
                for ko in range(8):
                    nc.sync.dma_start(out=Wv_t[:, ko, :], in_=Wv_r[:, ko, :])

                for ao in range(4):
                    for nt in range(2):
                        ps = psA.tile([128, 512], F32, tag="mm")
                        for ko in range(8):
                            nc.tensor.matmul(
                                ps, Wk_t[:, ko, ao * 128:(ao + 1) * 128],
                                iKT_t[:, ko, nt * 512:(nt + 1) * 512],
                                start=(ko == 0), stop=(ko == 7))
                        nc.vector.tensor_copy(
                            kT_t[:, ao, nt * 512:(nt + 1) * 512], ps)
                for mo in range(8):
                    ps = psA.tile([128, 512], F32, tag="mm")
                    for ko in range(8):
                        nc.tensor.matmul(
                            ps, iKT_t[:, ko, mo * 128:(mo + 1) * 128], Wv_t[:, ko, :],
                            start=(ko == 0), stop=(ko == 7))
                    nc.vector.tensor_copy(
                        v_t[:, mo, :, 0:64], ps.rearrange("p (h a) -> p h a", a=64))
                nc.vector.tensor_copy(
                    v_t[:, :, :, 64], ones_f32.rearrange("p (a b) -> p a b", a=8))

                # mask loads (needed from phase D on)
                mnT_r = mnT_h.ap().rearrange("(o p) q -> p o q", p=128)
                for so in range(8):
                    nc.sync.dma_start(out=mnT_t[:, so, :], in_=mnT_r[:, so, :])

                iQT_t = big_in.tile([128, 8, NQ], F32R, tag="bigin")
                Wq_t = wproj.tile([128, 8, HS], F32R, tag="w")
                iQT_r = iQT_h.ap().rearrange("(o p) q -> p o q", p=128)
                Wq_r = Wq_h.ap().rearrange("(o p) a -> p o a", p=128)
                for ko in range(8):
                    nc.sync.dma_start(out=iQT_t[:, ko, :], in_=iQT_r[:, ko, :])
                    nc.sync.dma_start(out=Wq_t[:, ko, :], in_=Wq_r[:, ko, :])
                for ao in range(4):
                    for nt in range(2):
                        ps = psA.tile([128, 512], F32, tag="mm")
                        for ko in range(8):
                            nc.tensor.matmul(
                                ps, Wq_t[:, ko, ao * 128:(ao + 1) * 128],
                                iQT_t[:, ko, nt * 512:(nt + 1) * 512],
                                start=(ko == 0), stop=(ko == 7))
                        nc.vector.tensor_copy(
                            qT_t[:, ao, nt * 512:(nt + 1) * 512], ps)

            with (
                tc.tile_pool(name="wo_pool", bufs=1) as wo_pool,
                tc.tile_pool(name="epool", bufs=3) as epool,
            ):
                Wo_t = wo_pool.tile([128, 4, D], F32R)
                nc.sync.dma_start(
                    out=Wo_t, in_=Wo_h.ap().rearrange("(o p) d -> p o d", p=128))

                # ---- phase D: head pairs (2ho, 2ho+1) x q-halves,
                # software-pipelined: iteration N's attn transposes/evicts are
                # emitted during iteration N+1 (recip chain is ready by then).
                def emit_scores(ho, qh):
                    q0 = qh * 512
                    eTs = [[epool.tile([128, 512], F16, tag=f"eT{i}_{so}",
                                       name=f"eT{i}_{so}")
                            for so in range(8)] for i in range(2)]
                    for so in range(8):
                        pss = [psA.tile([128, 512], F32, tag="mm", name="ps0"),
                               psA.tile([128, 512], F32, tag="mm", name="ps1")]
                        for ps in pss:
                            nc.tensor.matmul(
                                ps[0:64, :], ident_bf[0:64, 0:64],
                                mnT_t[0:64, so, q0:q0 + 512],
                                start=True, stop=False, tile_position=(0, 0))
                            nc.tensor.matmul(
                                ps[64:128, :], ident_bf[64:128, 64:128],
                                mnT_t[64:128, so, q0:q0 + 512],
                                start=True, stop=False, tile_position=(64, 64))
                        for i in range(2):
                            pb = 64 * i
                            nc.tensor.matmul(
                                pss[i], kT_t[pb:pb + 64, ho,
                                             so * 128:(so + 1) * 128],
                                qT_t[pb:pb + 64, ho, q0:q0 + 512],
                                start=False, stop=True, tile_position=(pb, 0))
                        for i in range(2):
                            nc.scalar.activation(
                                eTs[i][so], pss[i],
                                mybir.ActivationFunctionType.Exp, scale=0.125)
                    return eTs

                def emit_av(ho, qh, eTs):
                    rcs = []
                    chain = []
                    for i in range(2):
                        h = 2 * ho + i
                        eT_t = eTs[i]
                        po = psB.tile([65, 512], F32, tag="oT", name="po")
                        for so in range(8):
                            nc.tensor.matmul(
                                po, v_t[:, so, h, :], eT_t[so],
                                start=(so == 0), stop=(so == 7))
                        # quick-evict po so the PSUM bank frees immediately
                        po_s = small.tile([65, 512], F32R, tag="po_s",
                                          name="po_s")
                        nc.vector.tensor_copy(po_s, po)
                        rt = small.tile([128, 512], F32R, tag="rt", name="rt")
                        with nc.allow_low_precision(reason="f32r=4B fp32"):
                            nc.vector.reciprocal(rt[64:65, :], po_s[64:65, :])
                        # recip per-partition layout via DRAM bounce
                        scr = dpool.tile([512], F32R, tag="scr", name="scr")
                        nc.sync.dma_start(out=scr, in_=rt[64:65, :])
                        rc = small.tile([128, 4], F32R, tag="rc", name="rc",
                                        bufs=4)
                        nc.sync.dma_start(
                            out=rc, in_=scr.rearrange("(j p) -> p j", p=128))
                        rcs.append(rc)
                        chain.append((po_s, rt))
                    return rcs, chain

                def emit_ot_norm(ho, qh, chain):
                    q0 = qh * 512
                    for i in range(2):
                        po_s, rt = chain[i]
                        # broadcast recip along a for oT normalize
                        bc = psA.tile([128, 512], F32, tag="mm", name="bc")
                        nc.tensor.matmul(
                            bc[0:64, :], ones_t[64:65, :], rt[64:65, :],
                            start=True, stop=True)
                        oT_ev = small.tile([64, 512], F32R, tag="oT_ev",
                                           name="oT_ev")
                        nc.vector.tensor_mul(oT_ev, bc[0:64, :], po_s[0:64, :])
                        pb = 64 * i
                        nc.sync.dma_start(
                            out=oT_t[pb:pb + 64, ho, q0:q0 + 512], in_=oT_ev)

                def emit_attn_out(ho, qh, eTs, rcs):
                    q0 = qh * 512
                    for i in range(2):
                        h = 2 * ho + i
                        eT_t = eTs[i]
                        rc = rcs[i]
                        for qc in range(4):
                            for sh in range(2):
                                pt = psA.tile([128, 512], F16, tag="mm",
                                              name="pt")
                                for j in range(4):
                                    nc.tensor.transpose(
                                        pt[:, j * 128:(j + 1) * 128],
                                        eT_t[sh * 4 + j][:,
                                             qc * 128:(qc + 1) * 128],
                                        ident_f16)
                                at = stage.tile([128, 512], F16, tag="attn",
                                                name="at")
                                rc_f32 = rc[:, qc:qc + 1].bitcast(F32)
                                if (qc + 2 * sh) % 4 == 3:
                                    nc.scalar.mul(at, pt, rc_f32)
                                else:
                                    nc.vector.tensor_scalar_mul(at, pt, rc_f32)
                                nc.sync.dma_start(
                                    out=attn_h.ap()[
                                        h, q0 + qc * 128:q0 + (qc + 1) * 128,
                                        sh * 512:(sh + 1) * 512],
                                    in_=at)

                prev = None
                for ho in range(4):
                    for qh in range(2):
                        eTs = emit_scores(ho, qh)
                        if prev is not None:
                            pho, pqh, peTs, prcs, pchain = prev
                            emit_attn_out(pho, pqh, peTs, prcs)
                            emit_ot_norm(pho, pqh, pchain)
                        rcs, chain = emit_av(ho, qh, eTs)
                        prev = (ho, qh, eTs, rcs, chain)
                pho, pqh, peTs, prcs, pchain = prev
                emit_attn_out(pho, pqh, peTs, prcs)
                emit_ot_norm(pho, pqh, pchain)

                # ---- phase E: out_p = oT^T @ Wo
                for qt in range(8):
                    for nt in range(2):
                        ps = psA.tile([128, 512], F32, tag="mm")
                        for ko in range(4):
                            nc.tensor.matmul(
                                ps, oT_t[:, ko, qt * 128:(qt + 1) * 128],
                                Wo_t[:, ko, nt * 512:(nt + 1) * 512],
                                start=(ko == 0), stop=(ko == 3))
                        ot = stage.tile([128, 512], F32, tag="out", bufs=2)
                        nc.vector.tensor_copy(ot, ps)
                        nc.sync.dma_start(
                            out=outp_h.ap()[qt * 128:(qt + 1) * 128,
                                            nt * 512:(nt + 1) * 512],
                            in_=ot)

    nc.finalize()
    return nc


@functools.lru_cache(maxsize=1)
def _get_program():
    return _build_program()


def _shard_inputs(iQ, iK, mask, Wq, Wkv, Wo):
    iQ = np.asarray(iQ, dtype=np.float32)
    iK = np.asarray(iK, dtype=np.float32)
    mask = np.asarray(mask)
    Wq = np.asarray(Wq, dtype=np.float32)
    Wkv = np.asarray(Wkv, dtype=np.float32).reshape(D, 2, H, A)
    Wo = np.asarray(Wo, dtype=np.float32)
    ident = np.eye(128, dtype=np.float32)

    iQT = [np.ascontiguousarray(iQ[b].T) for b in range(B)]
    iKT = [np.ascontiguousarray(iK[b].T) for b in range(B)]
    import ml_dtypes
    mnT = [np.ascontiguousarray(
        (mask[b].T.astype(np.float32) * MASK_NEG).astype(ml_dtypes.float8_e5m2))
        for b in range(B)]

    in_maps = []
    for c in range(N_CORES):
        b, hh = c // 2, c % 2
        h0 = hh * HPC
        in_maps.append({
            "iQT": iQT[b],
            "iKT": iKT[b],
            "mnT": mnT[b],
            "Wq_s": np.ascontiguousarray(
                Wq.reshape(D, H, A)[:, h0:h0 + HPC].reshape(D, HS)),
            "Wk_s": np.ascontiguousarray(
                Wkv[:, 0, h0:h0 + HPC].reshape(D, HS)),
            "Wv_s": np.ascontiguousarray(
                Wkv[:, 1, h0:h0 + HPC].reshape(D, HS)),
            "Wo_s": np.ascontiguousarray(Wo[h0 * A:(h0 + HPC) * A, :]),
            "ident": ident,
        })
    return in_maps


def kernel(iQ, iK, mask, Wq, Wkv, Wo):
    nc = _get_program()
    in_maps = _shard_inputs(iQ, iK, mask, Wq, Wkv, Wo)
    res = run_bass_kernel_spmd(nc, in_maps, core_ids=list(range(N_CORES)))
    out = np.zeros((B, NQ, D), dtype=np.float32)
    attn = np.empty((B, H, NQ, SL), dtype=np.float32)
    for c in range(N_CORES):
        b, hh = c // 2, c % 2
        out[b] += res.results[c]["out_p"]
        attn[b, hh * HPC:(hh + 1) * HPC] = res.results[c]["attn_s"].astype(
            np.float32)
    return out, attn


# revision 27
# speedup vs baseline: 1.8979x; 1.1322x over previous
"""Cross-attention Trainium2 kernel (8-core SPMD, no collectives).

Problem: B=4, NQ=SL=D=1024, H=16, A=64.
  q = iQ @ Wq; k,v = iK @ Wkv; scores = q k^T / sqrt(A) masked; attn = softmax;
  out = (attn v) @ Wo.  Returns (out, attn).

Sharding: core c -> batch b=c//2, head-half hh=c%2 (8 heads each).
Host pre-transposes iQ/iK/mask per batch (pure data layout); the two
partial out projections per batch are summed on host.

Per-core dataflow (proj matmuls in float32r; eT/v in fp16):
  qT[a,q] = Wq_s^T iQ^T      kT[a,s] = Wk_s^T iK^T      v[s,a] = iK Wv_s
  scoresT[s,q] psum = maskT-copy (split identity, packed) + kT_h^T qT_h
      (head pairs packed on PE rows 0-63 / 64-127)
  eT = Exp(0.125 * psum)  fp16          (ACT, masked entries -> 0)
  oT_ext[a+1,q] += v_ext[s,a+1]^T eT    (ones column -> softmax denoms)
  attn[q,s] = PE-transpose(eT) * recip(denom)   (normalize on PSUM evict)
  out[q,D] = oT_norm^T Wo_s
"""
import functools
import numpy as np

import concourse.bass as bass
import concourse.mybir as mybir
import concourse.tile as tile
from concourse import bacc
from concourse.bass_utils import run_bass_kernel_spmd

B, NQ, SL, D = 4, 1024, 1024, 1024
H, A = 16, 64
HPC = 8            # heads per core
HS = HPC * A       # 512 = per-core slice of hidden
N_CORES = 8
F32 = mybir.dt.float32
F32R = mybir.dt.float32r
BF16 = mybir.dt.bfloat16
F16 = mybir.dt.float16
FP8 = mybir.dt.float8e5
MASK_NEG = -4096.0


def _build_program():
    nc = bacc.Bacc("TRN2", target_bir_lowering=False, debug=False)

    iQT_h = nc.dram_tensor("iQT", [D, NQ], F32R, kind="ExternalInput")
    iKT_h = nc.dram_tensor("iKT", [D, SL], F32R, kind="ExternalInput")
    mnT_h = nc.dram_tensor("mnT", [SL, NQ], FP8, kind="ExternalInput")
    Wq_h = nc.dram_tensor("Wq_s", [D, HS], F32R, kind="ExternalInput")
    Wk_h = nc.dram_tensor("Wk_s", [D, HS], F32R, kind="ExternalInput")
    Wv_h = nc.dram_tensor("Wv_s", [D, HS], F32R, kind="ExternalInput")
    Wo_h = nc.dram_tensor("Wo_s", [HS, D], F32R, kind="ExternalInput")
    ident_h = nc.dram_tensor("ident", [128, 128], F32R, kind="ExternalInput")
    attn_h = nc.dram_tensor("attn_s", [HPC, NQ, SL], F16, kind="ExternalOutput")
    outp_h = nc.dram_tensor("out_p", [NQ, D], F32, kind="ExternalOutput")

    with tile.TileContext(nc) as tc:
        with (
            tc.tile_pool(name="mpool", bufs=1) as mpool,       # maskT resident
            tc.tile_pool(name="actpool", bufs=1) as actpool,   # kT,qT,v_ext,oT resident
            tc.tile_pool(name="stage", bufs=6) as stage,       # attn/out staging
            tc.tile_pool(name="small", bufs=3) as small,
            tc.tile_pool(name="const", bufs=1) as cpool,
            tc.tile_pool(name="dpool", bufs=4, space="DRAM") as dpool,
            tc.tile_pool(name="psA", bufs=7, space="PSUM") as psA,
            tc.tile_pool(name="psB", bufs=1, space="PSUM") as psB,
        ):
            # ---- constants
            ident_t = cpool.tile([128, 128], F32R)
            nc.sync.dma_start(out=ident_t, in_=ident_h.ap())
            ones_f32 = cpool.tile([128, 64], F32)
            nc.vector.memset(ones_f32, 1.0)
            ones_t = cpool.tile([128, 64], F32R)
            nc.vector.tensor_copy(ones_t, ones_f32)
            ident_bf = cpool.tile([128, 128], FP8)
            nc.vector.tensor_copy(ident_bf, ident_t)
            ident_f16 = cpool.tile([128, 128], F16)
            nc.vector.tensor_copy(ident_f16, ident_t)

            mnT_t = mpool.tile([128, 8, NQ], FP8)
            kT_t = actpool.tile([128, 4, SL], F32R)       # [(2h,a), ho, s]
            v_t = actpool.tile([128, 8, HPC, 65], F16)    # [s_in, so, h, a+ones]
            qT_t = actpool.tile([128, 4, NQ], F32R)
            oT_t = actpool.tile([128, 4, NQ], F32R)       # packed normalized oT

            # ---- phases B/C: projections (input pools released afterwards)
            with (
                tc.tile_pool(name="big_in", bufs=2) as big_in,
                tc.tile_pool(name="wproj", bufs=2) as wproj,
            ):
                iKT_t = big_in.tile([128, 8, SL], F32R, tag="bigin")
                Wk_t = wproj.tile([128, 8, HS], F32R, tag="w")
                Wv_t = wproj.tile([128, 8, HS], F32R, tag="w")
                iKT_r = iKT_h.ap().rearrange("(o p) s -> p o s", p=128)
                Wk_r = Wk_h.ap().rearrange("(o p) a -> p o a", p=128)
                Wv_r = Wv_h.ap().rearrange("(o p) a -> p o a", p=128)
                for ko in range(8):
                    nc.sync.dma_start(out=iKT_t[:, ko, :], in_=iKT_r[:, ko, :])
                    nc.sync.dma_start(out=Wk_t[:, ko, :], in_=Wk_r[:, ko, :])
                for ko in range(8):
                    nc.sync.dma_start(out=Wv_t[:, ko, :], in_=Wv_r[:, ko, :])

                for ao in range(4):
                    for nt in range(2):
                        ps = psA.tile([128, 512], F32, tag="mm")
                        for ko in range(8):
                            nc.tensor.matmul(
                                ps, Wk_t[:, ko, ao * 128:(ao + 1) * 128],
                                iKT_t[:, ko, nt * 512:(nt + 1) * 512],
                                start=(ko == 0), stop=(ko == 7))
                        nc.vector.tensor_copy(
                            kT_t[:, ao, nt * 512:(nt + 1) * 512], ps)
                for mo in range(8):
                    ps = psA.tile([128, 512], F32, tag="mm")
                    for ko in range(8):
                        nc.tensor.matmul(
                            ps, iKT_t[:, ko, mo * 128:(mo + 1) * 128], Wv_t[:, ko, :],
                            start=(ko == 0), stop=(ko == 7))
                    nc.vector.tensor_copy(
                        v_t[:, mo, :, 0:64], ps.rearrange("p (h a) -> p h a", a=64))
                nc.vector.tensor_copy(
                    v_t[:, :, :, 64], ones_f32.rearrange("p (a b) -> p a b", a=8))

                # mask loads (needed from phase D on)
                mnT_r = mnT_h.ap().rearrange("(o p) q -> p o q", p=128)
                for so in range(8):
                    nc.sync.dma_start(out=mnT_t[:, so, :], in_=mnT_r[:, so, :])

                iQT_t = big_in.tile([128, 8, NQ], F32R, tag="bigin")
                Wq_t = wproj.tile([128, 8, HS], F32R, tag="w")
                iQT_r = iQT_h.ap().rearrange("(o p) q -> p o q", p=128)
                Wq_r = Wq_h.ap().rearrange("(o p) a -> p o a", p=128)
                for ko in range(8):
                    nc.sync.dma_start(out=iQT_t[:, ko, :], in_=iQT_r[:, ko, :])
                    nc.sync.dma_start(out=Wq_t[:, ko, :], in_=Wq_r[:, ko, :])
                for ao in range(4):
                    for nt in range(2):
                        ps = psA.tile([128, 512], F32, tag="mm")
                        for ko in range(8):
                            nc.tensor.matmul(
                                ps, Wq_t[:, ko, ao * 128:(ao + 1) * 128],
                                iQT_t[:, ko, nt * 512:(nt + 1) * 512],
                                start=(ko == 0), stop=(ko == 7))
                        nc.vector.tensor_copy(
                            qT_t[:, ao, nt * 512:(nt + 1) * 512], ps)

            with (
                tc.tile_pool(name="wo_pool", bufs=1) as wo_pool,
                tc.tile_pool(name="epool", bufs=3) as epool,
            ):
                Wo_t = wo_pool.tile([128, 4, D], F32R)
                nc.sync.dma_start(
                    out=Wo_t, in_=Wo_h.ap().rearrange("(o p) d -> p o d", p=128))

                # ---- phase D: head pairs (2ho, 2ho+1) x q-halves,
                # software-pipelined: iteration N's attn transposes/evicts are
                # emitted during iteration N+1 (recip chain is ready by then).
                def emit_scores(ho, qh):
                    q0 = qh * 512
                    eTs = [[epool.tile([128, 512], F16, tag=f"eT{i}_{so}",
                                       name=f"eT{i}_{so}")
                            for so in range(8)] for i in range(2)]
                    for so in range(8):
                        pss = [psA.tile([128, 512], F32, tag="mm", name="ps0"),
                               psA.tile([128, 512], F32, tag="mm", name="ps1")]
                        for ps in pss:
                            nc.tensor.matmul(
                                ps, ident_bf, mnT_t[:, so, q0:q0 + 512],
                                start=True, stop=False)
                        for i in range(2):
                            pb = 64 * i
                            nc.tensor.matmul(
                                pss[i], kT_t[pb:pb + 64, ho,
                                             so * 128:(so + 1) * 128],
                                qT_t[pb:pb + 64, ho, q0:q0 + 512],
                                start=False, stop=True, tile_position=(pb, 0))
                        for i in range(2):
                            nc.scalar.activation(
                                eTs[i][so], pss[i],
                                mybir.ActivationFunctionType.Exp, scale=0.125)
                    return eTs

                def emit_av(ho, qh, eTs):
                    rcs = []
                    chain = []
                    for i in range(2):
                        h = 2 * ho + i
                        eT_t = eTs[i]
                        po = psB.tile([65, 512], F32, tag="oT", name="po")
                        for so in range(8):
                            nc.tensor.matmul(
                                po, v_t[:, so, h, :], eT_t[so],
                                start=(so == 0), stop=(so == 7))
                        # quick-evict po so the PSUM bank frees immediately
                        po_s = small.tile([65, 512], F32R, tag="po_s",
                                          name="po_s")
                        nc.vector.tensor_copy(po_s, po)
                        rt = small.tile([128, 512], F32R, tag="rt", name="rt")
                        with nc.allow_low_precision(reason="f32r=4B fp32"):
                            nc.vector.reciprocal(rt[64:65, :], po_s[64:65, :])
                        # recip per-partition layout via DRAM bounce
                        scr = dpool.tile([512], F32R, tag="scr", name="scr")
                        nc.sync.dma_start(out=scr, in_=rt[64:65, :])
                        rc = small.tile([128, 4], F32R, tag="rc", name="rc",
                                        bufs=4)
                        nc.sync.dma_start(
                            out=rc, in_=scr.rearrange("(j p) -> p j", p=128))
                        rcs.append(rc)
                        chain.append((po_s, rt))
                    return rcs, chain

                def emit_ot_norm(ho, qh, chain):
                    q0 = qh * 512
                    for i in range(2):
                        po_s, rt = chain[i]
                        # broadcast recip along a for oT normalize
                        bc = psA.tile([128, 512], F32, tag="mm", name="bc")
                        nc.tensor.matmul(
                            bc[0:64, :], ones_t[64:65, :], rt[64:65, :],
                            start=True, stop=True)
                        oT_ev = small.tile([64, 512], F32R, tag="oT_ev",
                                           name="oT_ev")
                        nc.vector.tensor_mul(oT_ev, bc[0:64, :], po_s[0:64, :])
                        pb = 64 * i
                        nc.sync.dma_start(
                            out=oT_t[pb:pb + 64, ho, q0:q0 + 512], in_=oT_ev)

                def emit_attn_out(ho, qh, eTs, rcs):
                    q0 = qh * 512
                    for i in range(2):
                        h = 2 * ho + i
                        eT_t = eTs[i]
                        rc = rcs[i]
                        for qc in range(4):
                            for sh in range(2):
                                pt = psA.tile([128, 512], F16, tag="mm",
                                              name="pt")
                                for j in range(4):
                                    nc.tensor.transpose(
                                        pt[:, j * 128:(j + 1) * 128],
                                        eT_t[sh * 4 + j][:,
                                             qc * 128:(qc + 1) * 128],
                                        ident_f16)
                                at = stage.tile([128, 512], F16, tag="attn",
                                                name="at")
                                rc_f32 = rc[:, qc:qc + 1].bitcast(F32)
                                nc.vector.tensor_scalar_mul(at, pt, rc_f32)
                                nc.sync.dma_start(
                                    out=attn_h.ap()[
                                        h, q0 + qc * 128:q0 + (qc + 1) * 128,
                                        sh * 512:(sh + 1) * 512],
                                    in_=at)

                prev = None
                for ho in range(4):
                    for qh in range(2):
                        eTs = emit_scores(ho, qh)
                        if prev is not None:
                            pho, pqh, peTs, prcs, pchain = prev
                            emit_ot_norm(pho, pqh, pchain)
                            emit_attn_out(pho, pqh, peTs, prcs)
                        rcs, chain = emit_av(ho, qh, eTs)
                        prev = (ho, qh, eTs, rcs, chain)
                pho, pqh, peTs, prcs, pchain = prev
                emit_ot_norm(pho, pqh, pchain)
                emit_attn_out(pho, pqh, peTs, prcs)

                # ---- phase E: out_p = oT^T @ Wo
                for qt in range(8):
                    for nt in range(2):
                        ps = psA.tile([128, 512], F32, tag="mm")
                        for ko in range(4):
                            nc.tensor.matmul(
                                ps, oT_t[:, ko, qt * 128:(qt + 1) * 128],
                                Wo_t[:, ko, nt * 512:(nt + 1) * 512],
                                start=(ko == 0), stop=(ko == 3))
                        ot = stage.tile([128, 512], F32, tag="out", bufs=2)
                        nc.vector.tensor_copy(ot, ps)
                        nc.sync.dma_start(
                            out=outp_h.ap()[qt * 128:(qt + 1) * 128,
                                            nt * 512:(nt + 1) * 512],
                            in_=ot)

    nc.finalize()
    return nc


@functools.lru_cache(maxsize=1)
def _get_program():
    return _build_program()


def _shard_inputs(iQ, iK, mask, Wq, Wkv, Wo):
    iQ = np.asarray(iQ, dtype=np.float32)
    iK = np.asarray(iK, dtype=np.float32)
    mask = np.asarray(mask)
    Wq = np.asarray(Wq, dtype=np.float32)
    Wkv = np.asarray(Wkv, dtype=np.float32).reshape(D, 2, H, A)
    Wo = np.asarray(Wo, dtype=np.float32)
    ident = np.eye(128, dtype=np.float32)

    iQT = [np.ascontiguousarray(iQ[b].T) for b in range(B)]
    iKT = [np.ascontiguousarray(iK[b].T) for b in range(B)]
    import ml_dtypes
    mnT = [np.ascontiguousarray(
        (mask[b].T.astype(np.float32) * MASK_NEG).astype(ml_dtypes.float8_e5m2))
        for b in range(B)]

    in_maps = []
    for c in range(N_CORES):
        b, hh = c // 2, c % 2
        h0 = hh * HPC
        in_maps.append({
            "iQT": iQT[b],
            "iKT": iKT[b],
            "mnT": mnT[b],
            "Wq_s": np.ascontiguousarray(
                Wq.reshape(D, H, A)[:, h0:h0 + HPC].reshape(D, HS)),
            "Wk_s": np.ascontiguousarray(
                Wkv[:, 0, h0:h0 + HPC].reshape(D, HS)),
            "Wv_s": np.ascontiguousarray(
                Wkv[:, 1, h0:h0 + HPC].reshape(D, HS)),
            "Wo_s": np.ascontiguousarray(Wo[h0 * A:(h0 + HPC) * A, :]),
            "ident": ident,
        })
    return in_maps


def kernel(iQ, iK, mask, Wq, Wkv, Wo):
    nc = _get_program()
    in_maps = _shard_inputs(iQ, iK, mask, Wq, Wkv, Wo)
    res = run_bass_kernel_spmd(nc, in_maps, core_ids=list(range(N_CORES)))
    out = np.zeros((B, NQ, D), dtype=np.float32)
    attn = np.empty((B, H, NQ, SL), dtype=np.float32)
    for c in range(N_CORES):
        b, hh = c // 2, c % 2
        out[b] += res.results[c]["out_p"]
        attn[b, hh * HPC:(hh + 1) * HPC] = res.results[c]["attn_s"].astype(
            np.float32)
    return out, attn


# revision 29
# speedup vs baseline: 2.1628x; 1.1395x over previous
"""Cross-attention Trainium2 kernel (8-core SPMD, no collectives).

Problem: B=4, NQ=SL=D=1024, H=16, A=64.
  q = iQ @ Wq; k,v = iK @ Wkv; scores = q k^T / sqrt(A) masked; attn = softmax;
  out = (attn v) @ Wo.  Returns (out, attn).

Sharding: core c -> batch b=c//2, head-half hh=c%2 (8 heads each).
Host pre-transposes iQ/iK/mask per batch (pure data layout); the two
partial out projections per batch are summed on host.

Per-core dataflow (proj matmuls in float32r; eT/v in fp16):
  qT[a,q] = Wq_s^T iQ^T      kT[a,s] = Wk_s^T iK^T      v[s,a] = iK Wv_s
  scoresT[s,q] psum = maskT-copy (split identity, packed) + kT_h^T qT_h
      (head pairs packed on PE rows 0-63 / 64-127)
  eT = Exp(0.125 * psum)  fp16          (ACT, masked entries -> 0)
  oT_ext[a+1,q] += v_ext[s,a+1]^T eT    (ones column -> softmax denoms)
  attn[q,s] = PE-transpose(eT) * recip(denom)   (normalize on PSUM evict)
  out[q,D] = oT_norm^T Wo_s
"""
import functools
import numpy as np

import concourse.bass as bass
import concourse.mybir as mybir
import concourse.tile as tile
from concourse import bacc
from concourse.bass_utils import run_bass_kernel_spmd

B, NQ, SL, D = 4, 1024, 1024, 1024
H, A = 16, 64
HPC = 8            # heads per core
HS = HPC * A       # 512 = per-core slice of hidden
N_CORES = 8
F32 = mybir.dt.float32
F32R = mybir.dt.float32r
BF16 = mybir.dt.bfloat16
F16 = mybir.dt.float16
FP8 = mybir.dt.float8e5
MASK_NEG = -4096.0


def _build_program():
    nc = bacc.Bacc("TRN2", target_bir_lowering=False, debug=False)

    iQT_h = nc.dram_tensor("iQT", [D, NQ], F32R, kind="ExternalInput")
    iKT_h = nc.dram_tensor("iKT", [D, SL], F32R, kind="ExternalInput")
    mnT_h = nc.dram_tensor("mnT", [SL, NQ], FP8, kind="ExternalInput")
    Wq_h = nc.dram_tensor("Wq_s", [D, HS], F32R, kind="ExternalInput")
    Wk_h = nc.dram_tensor("Wk_s", [D, HS], F32R, kind="ExternalInput")
    Wv_h = nc.dram_tensor("Wv_s", [D, HS], F32R, kind="ExternalInput")
    Wo_h = nc.dram_tensor("Wo_s", [HS, D], F32R, kind="ExternalInput")
    ident_h = nc.dram_tensor("ident", [128, 128], F32R, kind="ExternalInput")
    attn_h = nc.dram_tensor("attn_s", [HPC, NQ, SL], F16, kind="ExternalOutput")
    outp_h = nc.dram_tensor("out_p", [NQ, D], F32, kind="ExternalOutput")

    with tile.TileContext(nc) as tc:
        with (
            tc.tile_pool(name="mpool", bufs=1) as mpool,       # maskT resident
            tc.tile_pool(name="actpool", bufs=1) as actpool,   # kT,qT,v_ext,oT resident
            tc.tile_pool(name="stage", bufs=6) as stage,       # attn/out staging
            tc.tile_pool(name="small", bufs=3) as small,
            tc.tile_pool(name="const", bufs=1) as cpool,
            tc.tile_pool(name="dpool", bufs=4, space="DRAM") as dpool,
            tc.tile_pool(name="psA", bufs=7, space="PSUM") as psA,
            tc.tile_pool(name="psB", bufs=1, space="PSUM") as psB,
        ):
            # ---- constants
            ident_t = cpool.tile([128, 128], F32R)
            nc.sync.dma_start(out=ident_t, in_=ident_h.ap())
            ones_f32 = cpool.tile([128, 64], F32)
            nc.vector.memset(ones_f32, 1.0)
            ones_t = cpool.tile([128, 64], F32R)
            nc.vector.tensor_copy(ones_t, ones_f32)
            ident_bf = cpool.tile([128, 128], FP8)
            nc.vector.tensor_copy(ident_bf, ident_t)
            ident_f16 = cpool.tile([128, 128], F16)
            nc.vector.tensor_copy(ident_f16, ident_t)

            mnT_t = mpool.tile([128, 8, NQ], FP8)
            kT_t = actpool.tile([128, 4, SL], F32R)       # [(2h,a), ho, s]
            v_t = actpool.tile([128, 8, HPC, 65], F16)    # [s_in, so, h, a+ones]
            qT_t = actpool.tile([128, 4, NQ], F32R)
            oT_t = actpool.tile([128, 4, NQ], F32R)       # packed normalized oT

            # ---- phases B/C: projections (input pools released afterwards)
            with (
                tc.tile_pool(name="big_in", bufs=2) as big_in,
                tc.tile_pool(name="wproj", bufs=2) as wproj,
            ):
                iKT_t = big_in.tile([128, 8, SL], F32R, tag="bigin")
                Wk_t = wproj.tile([128, 8, HS], F32R, tag="w")
                Wv_t = wproj.tile([128, 8, HS], F32R, tag="w")
                iKT_r = iKT_h.ap().rearrange("(o p) s -> p o s", p=128)
                Wk_r = Wk_h.ap().rearrange("(o p) a -> p o a", p=128)
                Wv_r = Wv_h.ap().rearrange("(o p) a -> p o a", p=128)
                for ko in range(8):
                    nc.sync.dma_start(out=iKT_t[:, ko, :], in_=iKT_r[:, ko, :])
                    nc.sync.dma_start(out=Wk_t[:, ko, :], in_=Wk_r[:, ko, :])
                for ko in range(8):
                    nc.sync.dma_start(out=Wv_t[:, ko, :], in_=Wv_r[:, ko, :])

                for ao in range(4):
                    for nt in range(2):
                        ps = psA.tile([128, 512], F32, tag="mm")
                        for ko in range(8):
                            nc.tensor.matmul(
                                ps, Wk_t[:, ko, ao * 128:(ao + 1) * 128],
                                iKT_t[:, ko, nt * 512:(nt + 1) * 512],
                                start=(ko == 0), stop=(ko == 7))
                        nc.vector.tensor_copy(
                            kT_t[:, ao, nt * 512:(nt + 1) * 512], ps)
                for mo in range(8):
                    ps = psA.tile([128, 512], F32, tag="mm")
                    for ko in range(8):
                        nc.tensor.matmul(
                            ps, iKT_t[:, ko, mo * 128:(mo + 1) * 128], Wv_t[:, ko, :],
                            start=(ko == 0), stop=(ko == 7))
                    nc.vector.tensor_copy(
                        v_t[:, mo, :, 0:64], ps.rearrange("p (h a) -> p h a", a=64))
                nc.vector.tensor_copy(
                    v_t[:, :, :, 64], ones_f32.rearrange("p (a b) -> p a b", a=8))

                # mask loads (needed from phase D on)
                mnT_r = mnT_h.ap().rearrange("(o p) q -> p o q", p=128)
                for so in range(8):
                    nc.sync.dma_start(out=mnT_t[:, so, :], in_=mnT_r[:, so, :])

                iQT_t = big_in.tile([128, 8, NQ], F32R, tag="bigin")
                Wq_t = wproj.tile([128, 8, HS], F32R, tag="w")
                iQT_r = iQT_h.ap().rearrange("(o p) q -> p o q", p=128)
                Wq_r = Wq_h.ap().rearrange("(o p) a -> p o a", p=128)
                for ko in range(8):
                    nc.sync.dma_start(out=iQT_t[:, ko, :], in_=iQT_r[:, ko, :])
                    nc.sync.dma_start(out=Wq_t[:, ko, :], in_=Wq_r[:, ko, :])
                for ao in range(4):
                    for nt in range(2):
                        ps = psA.tile([128, 512], F32, tag="mm")
                        for ko in range(8):
                            nc.tensor.matmul(
                                ps, Wq_t[:, ko, ao * 128:(ao + 1) * 128],
                                iQT_t[:, ko, nt * 512:(nt + 1) * 512],
                                start=(ko == 0), stop=(ko == 7))
                        nc.vector.tensor_copy(
                            qT_t[:, ao, nt * 512:(nt + 1) * 512], ps)

            with (
                tc.tile_pool(name="wo_pool", bufs=1) as wo_pool,
                tc.tile_pool(name="epool", bufs=3) as epool,
            ):
                Wo_t = wo_pool.tile([128, 4, D], F32R)
                nc.sync.dma_start(
                    out=Wo_t, in_=Wo_h.ap().rearrange("(o p) d -> p o d", p=128))

                # ---- phase D: head pairs (2ho, 2ho+1) x q-halves,
                # software-pipelined: iteration N's attn transposes/evicts are
                # emitted during iteration N+1 (recip chain is ready by then).
                def emit_scores(ho, qh):
                    q0 = qh * 512
                    eTs = [[epool.tile([128, 512], F16, tag=f"eT{i}_{so}",
                                       name=f"eT{i}_{so}")
                            for so in range(8)] for i in range(2)]
                    for so in range(8):
                        pss = [psA.tile([128, 512], F32, tag="mm", name="ps0"),
                               psA.tile([128, 512], F32, tag="mm", name="ps1")]
                        for ps in pss:
                            nc.tensor.matmul(
                                ps, ident_bf, mnT_t[:, so, q0:q0 + 512],
                                start=True, stop=False)
                        for i in range(2):
                            pb = 64 * i
                            nc.tensor.matmul(
                                pss[i], kT_t[pb:pb + 64, ho,
                                             so * 128:(so + 1) * 128],
                                qT_t[pb:pb + 64, ho, q0:q0 + 512],
                                start=False, stop=True, tile_position=(pb, 0))
                        for i in range(2):
                            nc.scalar.activation(
                                eTs[i][so], pss[i],
                                mybir.ActivationFunctionType.Exp, scale=0.125)
                    return eTs

                def emit_av(ho, qh, eTs):
                    rcs = []
                    chain = []
                    for i in range(2):
                        h = 2 * ho + i
                        eT_t = eTs[i]
                        po = psB.tile([65, 512], F32, tag="oT", name="po")
                        for so in range(8):
                            nc.tensor.matmul(
                                po, v_t[:, so, h, :], eT_t[so],
                                start=(so == 0), stop=(so == 7))
                        # quick-evict po so the PSUM bank frees immediately
                        po_s = small.tile([65, 512], F32R, tag="po_s",
                                          name="po_s")
                        nc.vector.tensor_copy(po_s, po)
                        rt = small.tile([128, 512], F32R, tag="rt", name="rt")
                        with nc.allow_low_precision(reason="f32r=4B fp32"):
                            nc.vector.reciprocal(rt[64:65, :], po_s[64:65, :])
                        # recip per-partition layout via DRAM bounce
                        scr = dpool.tile([512], F32R, tag="scr", name="scr")
                        nc.sync.dma_start(out=scr, in_=rt[64:65, :])
                        rc = small.tile([128, 4], F32R, tag="rc", name="rc",
                                        bufs=4)
                        nc.sync.dma_start(
                            out=rc, in_=scr.rearrange("(j p) -> p j", p=128))
                        rcs.append(rc)
                        chain.append((po_s, rt))
                    return rcs, chain

                def emit_ot_norm(ho, qh, chain):
                    q0 = qh * 512
                    for i in range(2):
                        po_s, rt = chain[i]
                        # broadcast recip along a for oT normalize
                        bc = psA.tile([128, 512], F32, tag="mm", name="bc")
                        nc.tensor.matmul(
                            bc[0:64, :], ones_t[64:65, :], rt[64:65, :],
                            start=True, stop=True)
                        oT_ev = small.tile([64, 512], F32R, tag="oT_ev",
                                           name="oT_ev")
                        nc.vector.tensor_mul(oT_ev, bc[0:64, :], po_s[0:64, :])
                        pb = 64 * i
                        nc.sync.dma_start(
                            out=oT_t[pb:pb + 64, ho, q0:q0 + 512], in_=oT_ev)

                def emit_attn_out(ho, qh, eTs, rcs):
                    q0 = qh * 512
                    for i in range(2):
                        h = 2 * ho + i
                        eT_t = eTs[i]
                        rc = rcs[i]
                        for qc in range(4):
                            at = stage.tile([128, 1024], F16, tag="attn",
                                            name="at")
                            rc_f32 = rc[:, qc:qc + 1].bitcast(F32)
                            for sh in range(2):
                                pt = psA.tile([128, 512], F16, tag="mm",
                                              name="pt")
                                for j in range(4):
                                    nc.tensor.transpose(
                                        pt[:, j * 128:(j + 1) * 128],
                                        eT_t[sh * 4 + j][:,
                                             qc * 128:(qc + 1) * 128],
                                        ident_f16)
                                nc.vector.tensor_scalar_mul(
                                    at[:, sh * 512:(sh + 1) * 512], pt, rc_f32)
                            nc.sync.dma_start(
                                out=attn_h.ap()[
                                    h, q0 + qc * 128:q0 + (qc + 1) * 128, :],
                                in_=at)

                prev = None
                for ho in range(4):
                    for qh in range(2):
                        eTs = emit_scores(ho, qh)
                        if prev is not None:
                            pho, pqh, peTs, prcs, pchain = prev
                            emit_ot_norm(pho, pqh, pchain)
                            emit_attn_out(pho, pqh, peTs, prcs)
                        rcs, chain = emit_av(ho, qh, eTs)
                        prev = (ho, qh, eTs, rcs, chain)
                pho, pqh, peTs, prcs, pchain = prev
                emit_ot_norm(pho, pqh, pchain)
                emit_attn_out(pho, pqh, peTs, prcs)

                # ---- phase E: out_p = oT^T @ Wo
                for qt in range(8):
                    ot = stage.tile([128, 1024], F32, tag="out", bufs=2)
                    for nt in range(2):
                        ps = psA.tile([128, 512], F32, tag="mm")
                        for ko in range(4):
                            nc.tensor.matmul(
                                ps, oT_t[:, ko, qt * 128:(qt + 1) * 128],
                                Wo_t[:, ko, nt * 512:(nt + 1) * 512],
                                start=(ko == 0), stop=(ko == 3))
                        nc.vector.tensor_copy(
                            ot[:, nt * 512:(nt + 1) * 512], ps)
                    nc.sync.dma_start(
                        out=outp_h.ap()[qt * 128:(qt + 1) * 128, :], in_=ot)

    nc.finalize()
    return nc


@functools.lru_cache(maxsize=1)
def _get_program():
    return _build_program()


def _shard_inputs(iQ, iK, mask, Wq, Wkv, Wo):
    iQ = np.asarray(iQ, dtype=np.float32)
    iK = np.asarray(iK, dtype=np.float32)
    mask = np.asarray(mask)
    Wq = np.asarray(Wq, dtype=np.float32)
    Wkv = np.asarray(Wkv, dtype=np.float32).reshape(D, 2, H, A)
    Wo = np.asarray(Wo, dtype=np.float32)
    ident = np.eye(128, dtype=np.float32)

    iQT = [np.ascontiguousarray(iQ[b].T) for b in range(B)]
    iKT = [np.ascontiguousarray(iK[b].T) for b in range(B)]
    import ml_dtypes
    mnT = [np.ascontiguousarray(
        (mask[b].T.astype(np.float32) * MASK_NEG).astype(ml_dtypes.float8_e5m2))
        for b in range(B)]

    in_maps = []
    for c in range(N_CORES):
        b, hh = c // 2, c % 2
        h0 = hh * HPC
        in_maps.append({
            "iQT": iQT[b],
            "iKT": iKT[b],
            "mnT": mnT[b],
            "Wq_s": np.ascontiguousarray(
                Wq.reshape(D, H, A)[:, h0:h0 + HPC].reshape(D, HS)),
            "Wk_s": np.ascontiguousarray(
                Wkv[:, 0, h0:h0 + HPC].reshape(D, HS)),
            "Wv_s": np.ascontiguousarray(
                Wkv[:, 1, h0:h0 + HPC].reshape(D, HS)),
            "Wo_s": np.ascontiguousarray(Wo[h0 * A:(h0 + HPC) * A, :]),
            "ident": ident,
        })
    return in_maps


def kernel(iQ, iK, mask, Wq, Wkv, Wo):
    nc = _get_program()
    in_maps = _shard_inputs(iQ, iK, mask, Wq, Wkv, Wo)
    res = run_bass_kernel_spmd(nc, in_maps, core_ids=list(range(N_CORES)))
    out = np.zeros((B, NQ, D), dtype=np.float32)
    attn = np.empty((B, H, NQ, SL), dtype=np.float32)
    for c in range(N_CORES):
        b, hh = c // 2, c % 2
        out[b] += res.results[c]["out_p"]
        attn[b, hh * HPC:(hh + 1) * HPC] = res.results[c]["attn_s"].astype(
            np.float32)
    return out, attn
